# revision 1
# baseline (speedup 1.0000x reference)
"""DCRNN diffusion-conv GNN forward on 8 trn2 NeuronCores.

Math (reference has H0=0, so the r-gate is dead and every dconv input is x):
  deg_out[v] = sum_{e:src=v} w[e]; deg_in[v] = sum_{e:dst=v} w[e]
  x_o = x / deg_out ; x_i = x / deg_in            (per-row scale)
  T_o1[d] = sum_{e:dst=d} x_o[src[e]]             (pure segment sums, coef
  T_i1[s] = sum_{e:src=s} x_i[dst[e]]              folded into the tables)
  T_o2 = segsum(T_o1/deg_out), T_i2 = segsum(T_i1/deg_in)
  G_g = x@(Wg[0,0]+Wg[1,0])[:32] + T_o1@Wg[0,1][:32] + T_i1@Wg[1,1][:32]
        + T_o2@Wg[0,2][:32] + T_i2@Wg[1,2][:32] + b_g      for g in {z,h}
  out = relu(sigmoid(-G_z) * tanh(G_h)) @ lin_w + lin_b

Distribution: nodes sharded 8 ways; edge lists partitioned by scatter-side
shard; gathers read replicated HBM tables (bf16, quad-row 256B descriptors);
segment sums are PE matmuls with DVE-built one-hot slot->node matrices into a
PSUM-resident [128, W*32] shard accumulator; shard T1 tables are exchanged
with AllGather. One SPMD program: per-(window,parity) slot budgets are maxed
across cores so the instruction stream is core-independent.
"""

import sys

sys.path.insert(0, "/opt/trn_rl_repo")

import numpy as np

N = 100000
C = 32
NCORES = 8
GATE = 64
OUTC = 32
CALL = 1024  # slots per dma_gather (2048+ risks SWDGE ring deadlock, 4096 confirmed hangs)


def _wrap_idx(a):
    # dma_gather index layout: idx i lives at partition i%16, col i//16,
    # replicated to all eight 16-partition groups.
    s = a.shape[0]
    w = a.reshape(s // 16, 16).T.astype(np.int16)
    return np.tile(w, (8, 1))


def _prep_dir(gnode, snode, wval, npad, sh):
    """Homogenized slot/chunk structure for one propagate direction.

    gnode: gather-side node per edge (global), snode: scatter-side node
    (global, defines core = snode//sh), wval: edge weight.
    Returns per-core device arrays + core-independent chunk metadata.
    """
    W = sh // 128
    core = snode // sh
    nl = snode - core * sh
    q = gnode % 4
    w = nl // 128
    cnt = np.zeros((NCORES, W, 4), np.int64)
    np.add.at(cnt, (core, w, q), 1)
    bud = 128 * np.ceil(cnt.max(axis=0) / 128).astype(np.int64)  # [W, 4]
    starts = np.concatenate([[0], np.cumsum(bud.reshape(-1))])[:-1].reshape(W, 4)
    S = int(bud.sum())
    S_pad = ((S + CALL - 1) // CALL) * CALL
    NV = S_pad // 128

    # slot position of each edge: starts[w,q] + rank within (core,w,q)
    key = (core * W + w) * 4 + q
    order = np.argsort(key, kind="stable")
    ranks = np.empty(len(key), np.int64)
    sk = key[order]
    brk = np.concatenate([[0], np.nonzero(np.diff(sk))[0] + 1])
    grp = np.zeros(len(sk), np.int64)
    grp[brk] = brk
    grp = np.maximum.accumulate(grp)
    ranks[order] = np.arange(len(sk)) - grp
    pos = starts[w, q] + ranks

    gidx_all, ldst_all, wgt_all = [], [], []
    for c in range(NCORES):
        m = core == c
        gidx = np.zeros(S_pad, np.int64)
        ldst = np.full(S_pad, -1.0, np.float32)
        wgt = np.zeros(S_pad, np.float32)
        gidx[pos[m]] = gnode[m] // 4
        ldst[pos[m]] = (nl[m] % 128).astype(np.float32)
        wgt[pos[m]] = wval[m]
        gidx_all.append(_wrap_idx(gidx))
        ldst_all.append(ldst.reshape(NV, 128).T.copy())
        wgt_all.append(wgt.reshape(NV, 128).T.copy())

    # chunk metadata (identical for every core)
    chunks = []  # (chunk_index, window, rhs_off, start, stop)
    for wi in range(W):
        cell = [(qi, k) for qi in range(4) for k in range(int(bud[wi, qi]) // 128)]
        for j, (qi, k) in enumerate(cell):
            ch = int(starts[wi, qi]) // 128 + k
            chunks.append((ch, wi, qi * 32, j == 0, j == len(cell) - 1))
    chunks.sort()
    return dict(
        S=S_pad, NV=NV, chunks=chunks,
        gidx=gidx_all, ldst=ldst_all, wgt=wgt_all,
    )


def _host_prep(x, edge_index, edge_weight):
    npad = ((N + 1024 * NCORES - 1) // (1024 * NCORES)) * 1024 * NCORES
    sh = npad // NCORES
    src = edge_index[0].astype(np.int64)
    dst = edge_index[1].astype(np.int64)
    wv = edge_weight.astype(np.float32)
    fwd = _prep_dir(src, dst, wv, npad, sh)  # scatter by dst, gather src
    rev = _prep_dir(dst, src, wv, npad, sh)  # scatter by src, gather dst
    x_pad = np.zeros((npad, C), np.float32)
    x_pad[:N] = x
    return npad, sh, fwd, rev, x_pad


def _build(npad, sh, fwd, rev, stop_after=None):
    import concourse.bacc as bacc
    import concourse.bass as bass
    import concourse.mybir as mybir
    import concourse.tile as tile

    W = sh // 128
    f32 = mybir.dt.float32
    bf16 = mybir.dt.bfloat16
    i16 = mybir.dt.int16
    AF = mybir.ActivationFunctionType
    OP = mybir.AluOpType
    RG = [list(range(NCORES))]

    nc = bacc.Bacc(target_bir_lowering=False)

    # ---------------- parameters ----------------
    x_rm = nc.declare_dram_parameter("x_rm", [npad, C], f32, isOutput=False)
    xT = nc.declare_dram_parameter("xT", [C, sh], f32, isOutput=False)
    io_bf = nc.declare_dram_parameter("io_bf", [128, 128], bf16, isOutput=False)
    id32 = nc.declare_dram_parameter("id32", [128, 128], f32, isOutput=False)
    wstk = nc.declare_dram_parameter("wstk", [6, 32, 128], f32, isOutput=False)
    bcat = nc.declare_dram_parameter("bcat", [128, 1], f32, isOutput=False)
    linw = nc.declare_dram_parameter("linw", [GATE, OUTC], f32, isOutput=False)
    linb = nc.declare_dram_parameter("linb", [OUTC, 1], f32, isOutput=False)
    pin = {}
    for nm, d in (("f", fwd), ("r", rev)):
        pin[nm + "idx"] = nc.declare_dram_parameter(f"{nm}idx", [128, d["S"] // 16], i16, isOutput=False)
        pin[nm + "ldst"] = nc.declare_dram_parameter(f"{nm}ldst", [128, d["NV"]], f32, isOutput=False)
        pin[nm + "wgt"] = nc.declare_dram_parameter(f"{nm}wgt", [128, d["NV"]], f32, isOutput=False)
    outT = nc.declare_dram_parameter("outT", [C, sh], f32, isOutput=True)

    # ---------------- internal DRAM ----------------
    def dram(name, shape, dt):
        return nc.dram_tensor(name, shape, dt)

    rin_flat = dram("rin_flat", [sh], f32)
    rout_flat = dram("rout_flat", [sh], f32)
    rin_full = dram("rin_full", [NCORES * sh], f32)
    rout_full = dram("rout_full", [NCORES * sh], f32)
    rpw_in = dram("rpw_in", [128, W], f32)
    rpw_out = dram("rpw_out", [128, W], f32)
    xo_tab = dram("xo_tab", [npad, C], bf16)
    xi_tab = dram("xi_tab", [npad, C], bf16)
    t1o_b = dram("t1o_b", [sh, C], bf16)
    t1i_b = dram("t1i_b", [sh, C], bf16)
    t1o_tab = dram("t1o_tab", [npad, C], bf16)
    t1i_tab = dram("t1i_tab", [npad, C], bf16)
    to1_raw = dram("to1_raw", [128, W * C], f32)
    ti1_raw = dram("ti1_raw", [128, W * C], f32)
    to2_raw = dram("to2_raw", [128, W * C], f32)

    TC = tile.TileContext

    # -------- helper: deg reduce pass (streamed weights, no gather) --------
    def deg_pass(tc, pool, spool, psum, d, wgt_par, ldst_par, iota):
        wbf = spool.tile([128, d["NV"]], bf16, tag="deg_wbf")
        wsb = spool.tile([128, d["NV"]], f32, tag="deg_wsb")
        lsb = spool.tile([128, d["NV"]], f32, tag="deg_lsb")
        nc.sync.dma_start(out=wsb[:], in_=wgt_par[:])
        nc.sync.dma_start(out=lsb[:], in_=ldst_par[:])
        nc.vector.tensor_copy(out=wbf[:], in_=wsb[:])
        dps = psum.tile([128, W], f32, space="PSUM")
        for ch, wi, off, st, sp in d["chunks"]:
            oh = pool.tile([128, 128], bf16, tag="deg_oh")
            nc.vector.tensor_scalar(
                out=oh[:], in0=iota[:], scalar1=lsb[:, ch : ch + 1],
                scalar2=None, op0=OP.is_equal,
            )
            nc.tensor.matmul(
                dps[:, wi : wi + 1], lhsT=oh[:], rhs=wbf[:, ch : ch + 1],
                start=st, stop=sp,
            )
        return dps, lsb

    # -------- helper: recip + row-major export --------
    def recip_export(tc, pool, psum, dps, rpw_dram, rflat_dram, ident):
        rec = pool.tile([128, W], f32, tag="rec")
        nc.vector.tensor_scalar(
            out=rec[:], in0=dps[:], scalar1=1e-20, scalar2=None, op0=OP.max,
        )
        nc.vector.reciprocal(out=rec[:], in_=rec[:])
        nc.sync.dma_start(out=rpw_dram[:], in_=rec[:])
        tp = psum.tile([W, 128], f32, space="PSUM")
        nc.tensor.transpose(out=tp[:], in_=rec[:], identity=ident[:])
        rm = pool.tile([W, 128], f32, tag="rm")
        nc.scalar.activation(out=rm[:], in_=tp[:], func=AF.Copy)
        nc.sync.dma_start(out=rflat_dram.rearrange("(w p) -> w p", p=128)[:], in_=rm[:])

    # -------- helper: scaled-table build (x * recip -> bf16 table) --------
    def build_table(tc, pool, rfull, tab):
        xv = x_rm.rearrange("(c p w) d -> c p (w d)", p=128, w=W)
        rv = rfull.rearrange("(c p w) -> c p w", p=128, w=W)
        tv = tab.rearrange("(c p w) d -> c p (w d)", p=128, w=W)
        for cc in range(NCORES):
            xt = pool.tile([128, W * C], f32, tag="xs_x")
            rt = pool.tile([128, W], f32, tag="xs_r")
            ot = pool.tile([128, W * C], bf16, tag="xs_o")
            nc.sync.dma_start(out=xt[:], in_=xv[cc])
            nc.sync.dma_start(out=rt[:], in_=rv[cc])
            nc.vector.tensor_tensor(
                out=ot[:].rearrange("p (w d) -> p w d", w=W),
                in0=xt[:].rearrange("p (w d) -> p w d", w=W),
                in1=rt[:].rearrange("p (w o) -> p w o", o=1).broadcast_to([128, W, C]),
                op=OP.mult,
            )
            nc.sync.dma_start(out=tv[cc], in_=ot[:])

    # -------- helper: one gather+reduce pass --------
    def hop_pass(tc, pool, spool, psum, d, idx_par, ldst_par, tab, iota):
        tabq = tab.rearrange("(q f) d -> q (f d)", f=4)
        isb = spool.tile([128, d["S"] // 16], i16, tag="hop_idx")
        lsb = spool.tile([128, d["NV"]], f32, tag="hop_ldst")
        nc.sync.dma_start(out=isb[:], in_=idx_par[:])
        nc.sync.dma_start(out=lsb[:], in_=ldst_par[:])
        acc = psum.tile([128, W * C], f32, space="PSUM")
        ncalls = d["S"] // CALL
        per = CALL // 128
        chmap = {}
        for e in d["chunks"]:
            chmap.setdefault(e[0] // per, []).append(e)
        for call in range(ncalls):
            gt = pool.tile([128, CALL // 128, 128], bf16, tag="hop_gt")
            nc.gpsimd.dma_gather(
                out_ap=gt[:],
                in_ap=tabq[:],
                idxs_ap=isb[:, call * (CALL // 16) : (call + 1) * (CALL // 16)],
                num_idxs=CALL,
                num_idxs_reg=CALL,
                elem_size=128,
            )
            for ch, wi, off, st, sp in chmap.get(call, []):
                j = ch % (CALL // 128)
                oh = pool.tile([128, 128], bf16, tag="hop_oh")
                nc.vector.tensor_scalar(
                    out=oh[:], in0=iota[:], scalar1=lsb[:, ch : ch + 1],
                    scalar2=None, op0=OP.is_equal,
                )
                nc.tensor.matmul(
                    acc[:, wi * C : (wi + 1) * C],
                    lhsT=oh[:],
                    rhs=gt[:, j, off : off + C],
                    start=st, stop=sp,
                )
        return acc

    # -------- helper: drain acc: raw f32 to dram, scaled bf16 to bounce ----
    def drain(tc, spool, acc, raw_dram, rpw_dram, bounce):
        tr = spool.tile([128, W * C], f32, tag="dr_raw")
        nc.vector.tensor_copy(out=tr[:], in_=acc[:])
        nc.sync.dma_start(out=raw_dram[:], in_=tr[:])
        if bounce is None:
            return
        rp = spool.tile([128, W], f32, tag="dr_rec")
        nc.sync.dma_start(out=rp[:], in_=rpw_dram[:])
        sc = spool.tile([128, W * C], bf16, tag="dr_sc")
        nc.vector.tensor_tensor(
            out=sc[:].rearrange("p (w d) -> p w d", w=W),
            in0=tr[:].rearrange("p (w d) -> p w d", w=W),
            in1=rp[:].rearrange("p (w o) -> p w o", o=1).broadcast_to([128, W, C]),
            op=OP.mult,
        )
        bv = bounce.rearrange("(w p) d -> p w d", p=128)
        nc.sync.dma_start(out=bv[:], in_=sc[:])

    def allgather(dst, srcb):
        return nc.gpsimd.collective_compute(
            "AllGather", OP.bypass, replica_groups=RG,
            ins=[srcb.ap().opt()], outs=[dst.ap().opt()],
        )

    # ================= CTX1: degrees =================
    with TC(nc) as tc:
        with (
            tc.tile_pool(name="p1", bufs=2) as pool,
            tc.tile_pool(name="ps1", bufs=1, space="PSUM") as psum,
            tc.tile_pool(name="c1", bufs=1) as cpool,
        ):
            iota = cpool.tile([128, 128], bf16)
            ident = cpool.tile([128, 128], f32)
            nc.sync.dma_start(out=iota[:], in_=io_bf[:])
            nc.sync.dma_start(out=ident[:], in_=id32[:])
            din, _ = deg_pass(tc, pool, cpool, psum, fwd, pin["fwgt"], pin["fldst"], iota)
            recip_export(tc, pool, psum, din, rpw_in, rin_flat, ident)
            dout, _ = deg_pass(tc, pool, cpool, psum, rev, pin["rwgt"], pin["rldst"], iota)
            recip_export(tc, pool, psum, dout, rpw_out, rout_flat, ident)

    with (
        nc.Block() as blk,
        nc.semaphore("cc1") as cc1,
    ):
        @blk.gpsimd
        def _(g):
            allgather(rin_full, rin_flat).then_inc(cc1, 1)
            g.wait_ge(cc1, 1)
            allgather(rout_full, rout_flat).then_inc(cc1, 1)
            g.wait_ge(cc1, 2)

    if stop_after == "ctx1":
        nc.compile()
        return nc

    # ================= CTX2: scaled x tables =================
    with TC(nc) as tc:
        with tc.tile_pool(name="p2", bufs=3) as pool:
            build_table(tc, pool, rout_full, xo_tab)
            build_table(tc, pool, rin_full, xi_tab)

    if stop_after == "ctx2":
        nc.compile()
        return nc

    # ================= CTX3: hop1 fwd =================
    with TC(nc) as tc:
        with (
            tc.tile_pool(name="p3", bufs=3) as pool,
            tc.tile_pool(name="ps3", bufs=1, space="PSUM") as psum,
            tc.tile_pool(name="c3", bufs=1) as cpool,
        ):
            iota = cpool.tile([128, 128], bf16)
            nc.sync.dma_start(out=iota[:], in_=io_bf[:])
            acc = hop_pass(tc, pool, cpool, psum, fwd, pin["fidx"], pin["fldst"], xo_tab, iota)
            drain(tc, cpool, acc, to1_raw, rpw_out, t1o_b)

    with (
        nc.Block() as blk2,
        nc.semaphore("cc2") as cc2,
    ):
        @blk2.gpsimd
        def _(g):
            allgather(t1o_tab, t1o_b).then_inc(cc2, 1)
            g.wait_ge(cc2, 1)

    if stop_after == "ctx3":
        nc.compile()
        return nc

    # ================= CTX4: hop1 rev =================
    with TC(nc) as tc:
        with (
            tc.tile_pool(name="p4", bufs=3) as pool,
            tc.tile_pool(name="ps4", bufs=1, space="PSUM") as psum,
            tc.tile_pool(name="c4", bufs=1) as cpool,
        ):
            iota = cpool.tile([128, 128], bf16)
            nc.sync.dma_start(out=iota[:], in_=io_bf[:])
            acc = hop_pass(tc, pool, cpool, psum, rev, pin["ridx"], pin["rldst"], xi_tab, iota)
            drain(tc, cpool, acc, ti1_raw, rpw_in, t1i_b)

    with (
        nc.Block() as blk3,
        nc.semaphore("cc3") as cc3,
    ):
        @blk3.gpsimd
        def _(g):
            allgather(t1i_tab, t1i_b).then_inc(cc3, 1)
            g.wait_ge(cc3, 1)

    if stop_after == "ctx4":
        nc.compile()
        return nc

    # ================= CTX5: hop2 fwd =================
    with TC(nc) as tc:
        with (
            tc.tile_pool(name="p5", bufs=3) as pool,
            tc.tile_pool(name="ps5", bufs=1, space="PSUM") as psum,
            tc.tile_pool(name="c5", bufs=1) as cpool,
        ):
            iota = cpool.tile([128, 128], bf16)
            nc.sync.dma_start(out=iota[:], in_=io_bf[:])
            acc = hop_pass(tc, pool, cpool, psum, fwd, pin["fidx"], pin["fldst"], t1o_tab, iota)
            drain(tc, cpool, acc, to2_raw, None, None)

    # ================= CTX6: hop2 rev + gates + output =================
    with TC(nc) as tc:
        with (
            tc.tile_pool(name="p6", bufs=3) as pool,
            tc.tile_pool(name="c6", bufs=1) as cpool,
        ):
            iota = cpool.tile([128, 128], bf16)
            ident = cpool.tile([128, 128], f32)
            nc.sync.dma_start(out=iota[:], in_=io_bf[:])
            nc.sync.dma_start(out=ident[:], in_=id32[:])
            ti2 = cpool.tile([128, W * C], f32)
            with tc.tile_pool(name="ps6", bufs=1, space="PSUM") as psum:
                acc = hop_pass(tc, pool, cpool, psum, rev, pin["ridx"], pin["rldst"], t1i_tab, iota)
                nc.vector.tensor_copy(out=ti2[:], in_=acc[:])
            psg_cm = tc.tile_pool(name="psg", bufs=2, space="PSUM")
            psg = psg_cm.__enter__()

            # F1 [128, sh]: rows 0:32 To1^T, 32:64 Ti1^T, 64:96 To2^T, 96:128 Ti2^T
            F1 = cpool.tile([128, sh], f32)
            for r, rawd in enumerate([to1_raw, ti1_raw, to2_raw]):
                tr = cpool.tile([128, W * C], f32, tag="ft_in")
                nc.sync.dma_start(out=tr[:], in_=rawd[:])
                for wi in range(W):
                    tp = psg.tile([C, 128], f32, space="PSUM", tag="ft_ps")
                    nc.tensor.transpose(
                        out=tp[:], in_=tr[:, wi * C : (wi + 1) * C], identity=ident[:]
                    )
                    nc.scalar.activation(
                        out=F1[r * C : (r + 1) * C, wi * 128 : (wi + 1) * 128],
                        in_=tp[:], func=AF.Copy,
                    )
            for wi in range(W):
                tp = psg.tile([C, 128], f32, space="PSUM", tag="ft_ps")
                nc.tensor.transpose(
                    out=tp[:], in_=ti2[:, wi * C : (wi + 1) * C], identity=ident[:]
                )
                nc.scalar.activation(
                    out=F1[3 * C : 4 * C, wi * 128 : (wi + 1) * 128], in_=tp[:], func=AF.Copy
                )

            # gate weights: W1 rows = [w(0,1), w(1,1), w(0,2), w(1,2)] blocks,
            # W2 = w(0,0)+w(1,0) (the x-term), matching F1 + streamed x^T
            W1 = cpool.tile([128, 128], f32)
            W2 = cpool.tile([C, 128], f32)
            wtmp = cpool.tile([C, 128], f32)
            for j in range(4):
                nc.sync.dma_start(out=W1[j * C : (j + 1) * C, :], in_=wstk[j + 2])
            nc.sync.dma_start(out=W2[:], in_=wstk[0])
            nc.sync.dma_start(out=wtmp[:], in_=wstk[1])
            nc.vector.tensor_tensor(out=W2[:], in0=W2[:], in1=wtmp[:], op=OP.add)
            nb = cpool.tile([128, 1], f32)
            nc.sync.dma_start(out=nb[:], in_=bcat[:])
            negb = cpool.tile([128, 1], f32)
            nc.vector.tensor_scalar(
                out=negb[:], in0=nb[:], scalar1=-1.0, scalar2=None, op0=OP.mult
            )
            lw = cpool.tile([GATE, OUTC], f32)
            lb = cpool.tile([OUTC, 1], f32)
            nc.sync.dma_start(out=lw[:], in_=linw[:])
            nc.sync.dma_start(out=lb[:], in_=linb[:])

            TILE = 512
            for t0 in range(0, sh, TILE):
                sl = slice(t0, t0 + TILE)
                xs = pool.tile([C, TILE], f32, tag="g_xs")
                nc.sync.dma_start(out=xs[:], in_=xT[:, sl])
                G = psg.tile([128, TILE], f32, space="PSUM", tag="g_ps")
                nc.tensor.matmul(G[:], lhsT=W1[:], rhs=F1[:, sl], start=True, stop=False)
                nc.tensor.matmul(G[:], lhsT=W2[:], rhs=xs[:], start=False, stop=True)
                zb = pool.tile([GATE, TILE], f32, tag="g_zb")
                ht = pool.tile([GATE, TILE], f32, tag="g_ht")
                nc.scalar.activation(
                    out=zb[:], in_=G[0:GATE, :], func=AF.Sigmoid,
                    bias=negb[0:GATE, :], scale=-1.0,
                )
                nc.scalar.activation(
                    out=ht[:], in_=G[GATE:128, :], func=AF.Tanh,
                    bias=nb[GATE:128, :], scale=1.0,
                )
                hs = pool.tile([GATE, TILE], f32, tag="g_hs")
                nc.vector.tensor_tensor(out=hs[:], in0=zb[:], in1=ht[:], op=OP.mult)
                hr = pool.tile([GATE, TILE], f32, tag="g_hr")
                nc.scalar.activation(out=hr[:], in_=hs[:], func=AF.Relu)
                po = psg.tile([OUTC, TILE], f32, space="PSUM", tag="o_ps")
                nc.tensor.matmul(po[:], lhsT=lw[:], rhs=hr[:], start=True, stop=True)
                ot = pool.tile([OUTC, TILE], f32, tag="g_ot")
                nc.vector.tensor_scalar(
                    out=ot[:], in0=po[:], scalar1=lb[:], scalar2=None, op0=OP.add
                )
                nc.sync.dma_start(out=outT[:, sl], in_=ot[:])
            psg_cm.__exit__(None, None, None)

    nc.compile()
    return nc


_CACHE = {}


def _get_built(x, edge_index, edge_weight):
    npad, sh, fwd, rev, x_pad = _host_prep(x, edge_index, edge_weight)
    nc = _build(npad, sh, fwd, rev)
    return npad, sh, fwd, rev, x_pad, nc


def kernel(x, edge_index, edge_weight, w_z, b_z, w_r, b_r, w_h, b_h, lin_w, lin_b):
    import ml_dtypes
    from concourse.bass_utils import run_bass_kernel_spmd

    x = np.asarray(x, np.float32)
    edge_index = np.asarray(edge_index)
    edge_weight = np.asarray(edge_weight, np.float32)
    import hashlib
    key = hashlib.sha1(
        np.ascontiguousarray(edge_index).tobytes()
        + np.ascontiguousarray(edge_weight).tobytes()
    ).hexdigest()
    if key not in _CACHE:
        _CACHE.clear()
        _CACHE[key] = _get_built(x, edge_index, edge_weight)
    npad, sh, fwd, rev, x_pad, nc = _CACHE[key]

    W = sh // 128
    iota = np.tile(np.arange(128, dtype=np.float32), (128, 1))
    wstk = np.zeros((6, 32, 128), np.float32)
    pairs = [(0, 0), (1, 0), (0, 1), (1, 1), (0, 2), (1, 2)]
    for j, (d, k) in enumerate(pairs):
        wstk[j, :, 0:64] = np.asarray(w_z, np.float32)[d, k, :32, :]
        wstk[j, :, 64:128] = np.asarray(w_h, np.float32)[d, k, :32, :]
    bcat = np.concatenate([np.asarray(b_z, np.float32), np.asarray(b_h, np.float32)])

    base = {
        "x_rm": x_pad,
        
        "io_bf": iota.astype(ml_dtypes.bfloat16),
        "id32": np.eye(128, dtype=np.float32),
        "wstk": wstk,
        "bcat": bcat.reshape(128, 1),
        "linw": np.asarray(lin_w, np.float32),
        "linb": np.asarray(lin_b, np.float32).reshape(OUTC, 1),
    }
    in_maps = []
    for c in range(NCORES):
        m = dict(base)
        m["xT"] = np.ascontiguousarray(x_pad.T[:, c * sh : (c + 1) * sh])
        m["fidx"] = fwd["gidx"][c]
        m["fldst"] = fwd["ldst"][c]
        m["fwgt"] = fwd["wgt"][c]
        m["ridx"] = rev["gidx"][c]
        m["rldst"] = rev["ldst"][c]
        m["rwgt"] = rev["wgt"][c]
        in_maps.append(m)

    import os
    trace = bool(int(os.environ.get("DCRNN_TRACE", "0")))
    res = run_bass_kernel_spmd(
        nc, in_maps, core_ids=list(range(NCORES)), trace=trace
    )
    global LAST_EXEC_NS
    LAST_EXEC_NS = res.exec_time_ns
    out = np.concatenate([res.results[c]["outT"] for c in range(NCORES)], axis=1)
    return np.ascontiguousarray(out.T[:N]).astype(np.float32)



# revision 50
# speedup vs baseline: 1.6312x; 1.6312x over previous
"""DCRNN diffusion-conv GNN forward on 8 trn2 NeuronCores.

Math (reference has H0=0, so the r-gate is dead and every dconv input is x):
  rpo[v] = 1/deg_out[v]; rpi[v] = 1/deg_in[v]      (host, graph-cached)
  To1[d] = sum_{e:dst=d} rpo[src]*x[src]           (hop1 fwd, weighted one-hot)
  Ti1[s] = sum_{e:src=s} rpi[dst]*x[dst]           (hop1 rev)
  To2[d] = sum_{e:dst=d} (rpo*To1)[src]            (hop2 fwd, scale folded into
  Ti2[s] = sum_{e:src=s} (rpi*Ti1)[dst]             the all-gathered table)
  G_g = x@(Wg[0,0]+Wg[1,0])[:32] + To1@Wg[0,1][:32] + Ti1@Wg[1,1][:32]
        + To2@Wg[0,2][:32] + Ti2@Wg[1,2][:32] + b_g      for g in {z,h}
  out = relu(sigmoid(-G_z) * tanh(G_h)) @ lin_w + lin_b

Device structure: nodes sharded 8 ways; edges partitioned by scatter-side
shard. Per edge one 256B quad-row descriptor gathers the source row from a
replicated bf16 HBM table; a DVE-built one-hot (slot -> local dst row) turns
the segment sum into PE matmuls into a channel-major PSUM accumulator
acc[32*(w%4)+c, (w//4)*128+p] so the gate matmuls consume it directly with no
transposes. Cell budgets are maxed across cores (SPMD-homogeneous instruction
stream) but NOT rounded up to the 128-slot chunk grid: chunks that straddle a
cell boundary run one matmul per overlapped cell using variant ldst arrays
(-1 = not-mine slots). Hop1 results are rescaled and AllGathered as the hop2
tables, with each collective hidden under the next hop's gather stream.
"""

import sys

sys.path.insert(0, "/opt/trn_rl_repo")

import numpy as np

N = 100000
C = 32
NCORES = 8
GATE = 64
OUTC = 32
CALL = 1024  # slots per dma_gather (2048+ risks SWDGE ring deadlock, 4096 confirmed hangs)


def _wrap16(a):
    # dma_gather index layout: idx i lives at partition i%16, col i//16,
    # replicated to all eight 16-partition groups (shipped pre-replicated).
    s = a.shape[0]
    w = a.reshape(s // 16, 16).T.astype(np.int16)
    return np.ascontiguousarray(np.tile(w, (8, 1)))


def _prep_dir(gnode, snode, coef, npad, sh):
    """Slot/chunk structure for one propagate direction.

    gnode: gather-side node per edge (global), snode: scatter-side node
    (global, defines core = snode//sh), coef: per-edge hop1 weight.
    Returns per-core device arrays + core-independent chunk metadata.
    """
    W = sh // 128
    core = snode // sh
    nl = snode - core * sh
    w = nl // 128
    r = (nl % 128).astype(np.float32)
    q = gnode % 4
    ncell = W * 4
    cellid = w * 4 + q
    cnt = np.zeros((NCORES, ncell), np.int64)
    np.add.at(cnt, (core, cellid), 1)
    bud = cnt.max(axis=0)  # [ncell], NOT rounded: chunks may straddle cells
    starts = np.concatenate([[0], np.cumsum(bud)])
    S = int(starts[-1])
    S_pad = ((S + CALL - 1) // CALL) * CALL
    NV = S_pad // 128

    # ordinal of each cell within its first chunk (0 unless the chunk also
    # contains the tail of earlier cells / starts of smaller ones)
    nz0 = np.concatenate([[0], np.cumsum(bud > 0)])
    c128 = (starts[:-1] // 128) * 128
    t = np.searchsorted(starts, c128, side="right")
    lo = np.maximum(t - 1, 0)
    ocell = nz0[np.arange(ncell)] - nz0[lo]
    ocell[bud == 0] = 0
    V = int(ocell.max()) + 1

    # slot position of each edge: starts[cell] + rank within (core,cell)
    key = core * ncell + cellid
    order = np.argsort(key, kind="stable")
    ranks = np.empty(len(key), np.int64)
    sk = key[order]
    brk = np.concatenate([[0], np.nonzero(np.diff(sk))[0] + 1])
    grp = np.zeros(len(sk), np.int64)
    grp[brk] = brk
    grp = np.maximum.accumulate(grp)
    ranks[order] = np.arange(len(sk)) - grp
    pos = starts[cellid] + ranks
    var = np.where(pos // 128 == starts[cellid] // 128, ocell[cellid], 0)

    gidx = np.zeros((NCORES, S_pad), np.int64)
    ldst = np.full((NCORES, V, S_pad), -1.0, np.float32)
    wgt = np.zeros((NCORES, S_pad), np.float32)
    gidx[core, pos] = gnode // 4
    ldst[core, var, pos] = r
    wgt[core, pos] = coef

    # chunk segments (identical for every core): (chunk, variant, wi, qoff)
    segs = []
    for i in range(ncell):
        if bud[i] == 0:
            continue
        s, e = int(starts[i]), int(starts[i + 1])
        wi, qi = i // 4, i % 4
        for ch in range(s // 128, (e - 1) // 128 + 1):
            v = int(ocell[i]) if ch == s // 128 else 0
            segs.append((ch, v, wi, qi))
    segs.sort()
    first, last = {}, {}
    for idx, (ch, v, wi, qi) in enumerate(segs):
        if wi not in first:
            first[wi] = idx
        last[wi] = idx
    assert len(first) == W, "empty window: add dummy-segment support"
    chunks = [
        (ch, v, wi, qi * 32, idx == first[wi], idx == last[wi])
        for idx, (ch, v, wi, qi) in enumerate(segs)
    ]
    return dict(
        S=S_pad, NV=NV, V=V, chunks=chunks,
        gidx=[_wrap16(gidx[c]) for c in range(NCORES)],
        ldst=[np.ascontiguousarray(ldst[c].reshape(V, NV, 128).transpose(0, 2, 1))
              for c in range(NCORES)],
        wgt=[np.ascontiguousarray(wgt[c].reshape(NV, 128).T) for c in range(NCORES)],
    )


def _host_prep(x, edge_index, edge_weight):
    npad = ((N + 512 * NCORES - 1) // (512 * NCORES)) * 512 * NCORES
    sh = npad // NCORES
    src = edge_index[0].astype(np.int64)
    dst = edge_index[1].astype(np.int64)
    wv = edge_weight.astype(np.float64)
    deg_out = np.bincount(src, weights=wv, minlength=npad)
    deg_in = np.bincount(dst, weights=wv, minlength=npad)
    rpo = np.where(deg_out > 0, 1.0 / np.maximum(deg_out, 1e-300), 0.0).astype(np.float32)
    rpi = np.where(deg_in > 0, 1.0 / np.maximum(deg_in, 1e-300), 0.0).astype(np.float32)
    fwd = _prep_dir(src, dst, rpo[src], npad, sh)  # scatter by dst, gather src
    rev = _prep_dir(dst, src, rpi[dst], npad, sh)  # scatter by src, gather dst
    JW = sh // 512
    # drain rescale tables, p-major: rpw[p, j*4+q] = rp[core*sh + (4j+q)*128 + p]
    rpw_o = [np.ascontiguousarray(
        rpo[c * sh:(c + 1) * sh].reshape(JW, 4, 128).transpose(2, 0, 1).reshape(128, JW * 4))
        for c in range(NCORES)]
    rpw_i = [np.ascontiguousarray(
        rpi[c * sh:(c + 1) * sh].reshape(JW, 4, 128).transpose(2, 0, 1).reshape(128, JW * 4))
        for c in range(NCORES)]
    return npad, sh, fwd, rev, rpw_o, rpw_i


def _build(npad, sh, fwd, rev):
    import concourse.bacc as bacc
    import concourse.bass as bass
    import concourse.mybir as mybir
    import concourse.tile as tile

    W = sh // 128
    JW = W // 4
    AC = JW * 128  # accumulator free dim
    # j-chunks of the accumulator, drained as they complete; the tail is the
    # last chunk's gate work, so later chunks are finer
    JSPLIT = [JW - 18, 6, 6, 3, 3]
    J0 = [0] + list(np.cumsum(JSPLIT)[:-1])
    NSEG = len(JSPLIT)
    QCOL = [j * 128 for j in JSPLIT]
    f32 = mybir.dt.float32
    bf16 = mybir.dt.bfloat16
    i16 = mybir.dt.int16
    AF = mybir.ActivationFunctionType
    OP = mybir.AluOpType
    RG = [list(range(NCORES))]

    def quarter_of(j):
        for k in range(NSEG - 1, -1, -1):
            if j >= J0[k]:
                return k

    def tiles_of(jn):
        cols = jn * 128
        out = []
        c0 = 0
        while cols > 0:
            tw = 512 if cols >= 512 else cols
            out.append((c0, tw))
            c0 += tw
            cols -= tw
        return out

    nc = bacc.Bacc(target_bir_lowering=False)

    # ---------------- parameters ----------------
    xbf = nc.declare_dram_parameter("xbf", [npad, C], bf16, isOutput=False)
    xacc = nc.declare_dram_parameter("xacc", [128, AC], bf16, isOutput=False)
    io_bf = nc.declare_dram_parameter("io_bf", [128, 128], bf16, isOutput=False)
    idbf = nc.declare_dram_parameter("idbf", [128, 128], bf16, isOutput=False)
    wsum4 = nc.declare_dram_parameter("wsum4", [128, 128], bf16, isOutput=False)
    wblk4 = nc.declare_dram_parameter("wblk4", [4, 128, 128], bf16, isOutput=False)
    bcat = nc.declare_dram_parameter("bcat", [128, 1], f32, isOutput=False)
    nbneg = nc.declare_dram_parameter("nbneg", [128, 1], f32, isOutput=False)
    linw = nc.declare_dram_parameter("linw", [GATE, OUTC], bf16, isOutput=False)
    linb = nc.declare_dram_parameter("linb", [OUTC, 1], f32, isOutput=False)
    pin = {}
    for nm, d in (("f", fwd), ("r", rev)):
        pin[nm + "idx"] = nc.declare_dram_parameter(f"{nm}idx", [128, d["S"] // 16], i16, isOutput=False)
        pin[nm + "ldst"] = nc.declare_dram_parameter(f"{nm}ldst", [d["V"], 128, d["NV"]], f32, isOutput=False)
        pin[nm + "wgt"] = nc.declare_dram_parameter(f"{nm}wgt", [128, d["NV"]], f32, isOutput=False)
    rpwo = nc.declare_dram_parameter("rpwo", [128, JW * 4], f32, isOutput=False)
    rpwi = nc.declare_dram_parameter("rpwi", [128, JW * 4], f32, isOutput=False)
    outT = nc.declare_dram_parameter("outT", [OUTC, sh], f32, isOutput=True)

    TC = tile.TileContext

    def load_idx(pool, d, par, tag, chunks=1):
        isb = pool.tile([128, d["S"] // 16], i16, tag=tag, name=tag)
        n = d["S"] // 16
        step = -(-n // chunks)
        for c0 in range(0, n, step):
            c1 = min(c0 + step, n)
            nc.sync.dma_start(out=isb[:, c0:c1], in_=par[:, c0:c1])
        return isb

    def load_ldst(pool, d, par, tag):
        lsb = pool.tile([128, d["V"] * d["NV"]], f32, tag=tag, name=tag)
        for v in range(d["V"]):
            nc.sync.dma_start(
                out=lsb[:, v * d["NV"]:(v + 1) * d["NV"]], in_=par[v],
            )
        return lsb

    # -------- one gather+segment-sum pass (quarter-split accumulator) ------
    def hop_pass(pool, psum, d, isb, lsb, wsb, tab, iota, on_call_end=None):
        tabq = tab.rearrange("(g f) d -> g (f d)", f=4)
        accs = [
            psum.tile([128, QCOL[k]], f32, space="PSUM", tag="acc", name="acc")
            for k in range(NSEG)
        ]
        per = CALL // 128
        chmap = {}
        for e in d["chunks"]:
            chmap.setdefault(e[0] // per, []).append(e)
        NVd = d["NV"]
        for call in range(d["S"] // CALL):
            gt = pool.tile([128, per, 128], bf16, tag="gt", bufs=10, name="gt")
            nc.gpsimd.dma_gather(
                out_ap=gt[:],
                in_ap=tabq[:],
                idxs_ap=isb[:, call * (CALL // 16):(call + 1) * (CALL // 16)],
                num_idxs=CALL,
                num_idxs_reg=CALL,
                elem_size=128,
            )
            for ch, v, wi, qoff, st, sp in chmap.get(call, []):
                j = ch % per
                oh = pool.tile([128, 128], bf16, tag="oh", bufs=16, name="oh")
                lcol = lsb[:, v * NVd + ch: v * NVd + ch + 1]
                if wsb is None:
                    nc.vector.tensor_scalar(
                        out=oh[:], in0=iota[:], scalar1=lcol, scalar2=None,
                        op0=OP.is_equal,
                    )
                else:
                    nc.vector.tensor_scalar(
                        out=oh[:], in0=iota[:], scalar1=lcol,
                        scalar2=wsb[:, ch:ch + 1], op0=OP.is_equal, op1=OP.mult,
                    )
                wj = wi // 4
                k = quarter_of(wj)
                nc.tensor.matmul(
                    accs[k][32 * (wi % 4):32 * (wi % 4) + 32,
                            (wj - J0[k]) * 128:(wj - J0[k]) * 128 + 128],
                    lhsT=gt[:, j, qoff:qoff + 32],
                    rhs=oh[:],
                    start=st, stop=sp,
                    tile_position=(0, 32 * (wi % 4)),
                )
            if on_call_end is not None:
                on_call_end(call, accs)
        return accs

    # -------- drain one quarter: bf16 raster; rescaled node-major bounce ---
    def drain_q(cpool, psumt, acc_k, k, raster_slice, rp, sc, bounce, identbf):
        nc.vector.tensor_copy(out=raster_slice, in_=acc_k[:])
        if bounce is None:
            return
        j0, jn = J0[k], JSPLIT[k]
        for j in range(j0, j0 + jn):
            tp = psumt.tile([128, 128], bf16, space="PSUM", tag="dr_ps", name="dr_ps")
            nc.tensor.transpose(
                out=tp[:], in_=raster_slice[:, (j - j0) * 128:(j - j0 + 1) * 128],
                identity=identbf[:],
            )
            nc.vector.tensor_tensor(
                out=sc[:, j * 128:(j + 1) * 128].rearrange("p (q c) -> p q c", q=4),
                in0=tp[:].rearrange("p (q c) -> p q c", q=4),
                in1=rp[:, j * 4:(j + 1) * 4].rearrange("p (q o) -> p q o", o=1)
                    .broadcast_to([128, 4, C]),
                op=OP.mult,
            )
        nc.sync.dma_start(
            out=bounce.rearrange("(j q p) c -> p j q c", q=4, p=128)[:, j0:j0 + jn],
            in_=sc[:, j0 * 128:(j0 + jn) * 128]
                .rearrange("p (j q c) -> p j q c", j=jn, q=4),
        )

    # ================= single context =================
    with TC(nc) as tc:
        with (
            tc.tile_pool(name="p", bufs=3) as pool,
            tc.tile_pool(name="c", bufs=1) as cpool,
            tc.tile_pool(name="w", bufs=2) as wpool,
            tc.tile_pool(name="d", bufs=1, space="DRAM") as dpool,
            tc.tile_pool(name="ps", bufs=2, space="PSUM") as psum,
            tc.tile_pool(name="pst", bufs=1, space="PSUM") as psumt,
            tc.tile_pool(name="psg", bufs=1, space="PSUM") as psg,
        ):
            t1o_b = dpool.tile([sh, C], bf16, name="t1o_b")
            t1i_b = dpool.tile([sh, C], bf16, name="t1i_b")
            t1o_tab = dpool.tile([npad, C], bf16, name="t1o_tab")
            t1i_tab = dpool.tile([npad, C], bf16, name="t1i_tab")

            iota = cpool.tile([128, 128], bf16, name="iota")
            identbf = cpool.tile([128, 128], bf16, name="identbf")
            # critical-path loads first: pass-1 gathers need only iota+isb_f
            nc.sync.dma_start(out=iota[:], in_=io_bf[:])
            isb_f = load_idx(cpool, fwd, pin["fidx"], "isb_f")
            lsb_f = load_ldst(cpool, fwd, pin["fldst"], "lsb_f")
            wsb_f = wpool.tile([128, fwd["NV"]], f32, tag="wgt", name="wsb_f")
            nc.sync.dma_start(out=wsb_f[:], in_=pin["fwgt"][:])
            nc.sync.dma_start(out=identbf[:], in_=idbf[:])

            to1_sb = cpool.tile([128, AC], bf16, name="to1_sb")
            ti1_sb = cpool.tile([128, AC], bf16, name="ti1_sb")
            to2_sb = cpool.tile([128, AC], bf16, name="to2_sb")
            ti2_k = [
                cpool.tile([128, QCOL[k]], bf16, tag=f"ti2_{k}", name=f"ti2_{k}")
                for k in range(NSEG)
            ]
            xacc_sb = cpool.tile([128, AC], bf16, name="xacc_sb")
            ws = cpool.tile([128, 128], bf16, name="ws")
            wb = cpool.tile([128, 4, 128], bf16, name="wb")
            nb = cpool.tile([128, 1], f32, name="nb")
            ngb = cpool.tile([128, 1], f32, name="ngb")
            lw = cpool.tile([GATE, OUTC], bf16, name="lw")
            lb = cpool.tile([OUTC, 1], f32, name="lb")
            rpo_sb = cpool.tile([128, JW * 4], f32, name="rpo_sb")
            rpi_sb = cpool.tile([128, JW * 4], f32, name="rpi_sb")
            sc = cpool.tile([128, AC], bf16, tag="dr_sc", name="dr_sc")

            # ---------------- pass 1: hop1 fwd ----------------
            accs = hop_pass(pool, psum, fwd, isb_f, lsb_f, wsb_f, xbf, iota)
            # off-critical-path loads overlap the pass-1 gather stream
            isb_r = load_idx(cpool, rev, pin["ridx"], "isb_r", chunks=6)
            lsb_r = load_ldst(cpool, rev, pin["rldst"], "lsb_r")
            wsb_r = wpool.tile([128, rev["NV"]], f32, tag="wgt", name="wsb_r")
            nc.sync.dma_start(out=wsb_r[:], in_=pin["rwgt"][:])
            nc.sync.dma_start(out=xacc_sb[:], in_=xacc[:])
            nc.sync.dma_start(out=ws[:], in_=wsum4[:])
            nc.sync.dma_start(out=wb[:], in_=wblk4.rearrange("r p n -> p r n"))
            nc.sync.dma_start(out=nb[:], in_=bcat[:])
            nc.sync.dma_start(out=ngb[:], in_=nbneg[:])
            nc.sync.dma_start(out=lw[:], in_=linw[:])
            nc.sync.dma_start(out=lb[:], in_=linb[:])
            nc.sync.dma_start(out=rpo_sb[:], in_=rpwo[:])
            nc.sync.dma_start(out=rpi_sb[:], in_=rpwi[:])

            for k in range(NSEG):
                drain_q(cpool, psumt, accs[k], k,
                        to1_sb[:, J0[k] * 128:(J0[k] + JSPLIT[k]) * 128],
                        rpo_sb, sc, t1o_b, identbf)
            nc.gpsimd.collective_compute(
                "AllGather", OP.bypass, replica_groups=RG,
                ins=[t1o_b.opt()], outs=[t1o_tab.opt()],
            )

            # ---------------- pass 2: hop1 rev ----------------
            accs = hop_pass(pool, psum, rev, isb_r, lsb_r, wsb_r, xbf, iota)
            for k in range(NSEG):
                drain_q(cpool, psumt, accs[k], k,
                        ti1_sb[:, J0[k] * 128:(J0[k] + JSPLIT[k]) * 128],
                        rpi_sb, sc, t1i_b, identbf)
            nc.gpsimd.collective_compute(
                "AllGather", OP.bypass, replica_groups=RG,
                ins=[t1i_b.opt()], outs=[t1i_tab.opt()],
            )

            # ---------------- pass 3: hop2 fwd ----------------
            accs = hop_pass(pool, psum, fwd, isb_f, lsb_f, None, t1o_tab, iota)
            for k in range(NSEG):
                drain_q(cpool, psumt, accs[k], k,
                        to2_sb[:, J0[k] * 128:(J0[k] + JSPLIT[k]) * 128],
                        None, None, None, identbf)

            # ---------------- pass 4: hop2 rev + gates -------------------
            # quarter drains land in ti2_k[k]; gate tiles for quarter k start
            # as soon as it drains, hiding most of the gate phase under the
            # hop2r gather stream.
            outv = outT.rearrange("c (j q p) -> q c j p", q=4, p=128)
            per = CALL // 128
            qlast = {}
            for e in rev["chunks"]:
                kq = quarter_of(e[2] // 4)
                qlast[kq] = max(qlast.get(kq, -1), e[0] // per)
            ncalls_r = rev["S"] // CALL

            def emit_quarter(k, accs):
                nc.vector.tensor_copy(out=ti2_k[k][:], in_=accs[k][:])
                j0, jn = J0[k], JSPLIT[k]
                for qi in range(4):
                    sl4 = slice(32 * qi, 32 * qi + 32)
                    ot = pool.tile([OUTC, max(QCOL)], f32, tag="g_ot", bufs=2, name="ot")
                    for c0, TW in tiles_of(jn):
                        gsl = slice(j0 * 128 + c0, j0 * 128 + c0 + TW)
                        G = psg.tile([128, 512], f32, space="PSUM", tag="G",
                                     bufs=2, name="G")
                        terms = [
                            (ws[sl4, :], xacc_sb[sl4, gsl]),
                            (wb[sl4, 0, :], to1_sb[sl4, gsl]),
                            (wb[sl4, 1, :], ti1_sb[sl4, gsl]),
                            (wb[sl4, 2, :], to2_sb[sl4, gsl]),
                            (wb[sl4, 3, :], ti2_k[k][sl4, c0:c0 + TW]),
                        ]
                        for ti, (lhsT, rhs) in enumerate(terms):
                            nc.tensor.matmul(
                                G[:, 0:TW], lhsT=lhsT, rhs=rhs,
                                start=(ti == 0), stop=(ti == len(terms) - 1),
                                tile_position=(32 * qi, 0),
                            )
                        zb = pool.tile([GATE, 512], bf16, tag="g_zb", name="zb")
                        ht = pool.tile([GATE, 512], bf16, tag="g_ht", name="ht")
                        nc.scalar.activation(
                            out=zb[:, 0:TW], in_=G[0:GATE, 0:TW], func=AF.Sigmoid,
                            bias=ngb[0:GATE, :], scale=-1.0,
                        )
                        nc.scalar.activation(
                            out=ht[:, 0:TW], in_=G[GATE:128, 0:TW], func=AF.Tanh,
                            bias=nb[GATE:128, :], scale=1.0,
                        )
                        hs = pool.tile([GATE, 512], bf16, tag="g_hs", name="hs")
                        nc.vector.tensor_tensor(
                            out=hs[:, 0:TW], in0=zb[:, 0:TW], in1=ht[:, 0:TW],
                            op=OP.mult,
                        )
                        hr = pool.tile([GATE, 512], bf16, tag="g_hr", name="hr")
                        nc.vector.tensor_scalar(
                            out=hr[:, 0:TW], in0=hs[:, 0:TW], scalar1=0.0,
                            scalar2=None, op0=OP.max,
                        )
                        po = psg.tile([OUTC, 512], f32, space="PSUM", tag="po",
                                      bufs=1, name="po")
                        nc.tensor.matmul(
                            po[:, 0:TW], lhsT=lw[:], rhs=hr[:, 0:TW],
                            start=True, stop=True,
                        )
                        nc.vector.tensor_scalar(
                            out=ot[:, c0:c0 + TW], in0=po[:, 0:TW], scalar1=lb[:],
                            scalar2=None, op0=OP.add,
                        )
                    nc.sync.dma_start(
                        out=outv[qi][:, j0:j0 + jn, :],
                        in_=ot[:, 0:jn * 128].rearrange("c (j p) -> c j p", p=128),
                    )

            done = set()

            def on_call_end(call, accs):
                for k in range(NSEG):
                    if qlast.get(k) == call and k not in done:
                        done.add(k)
                        emit_quarter(k, accs)

            hop_pass(pool, psum, rev, isb_r, lsb_r, None, t1i_tab, iota,
                     on_call_end=on_call_end)
            for k in range(NSEG):
                if k not in done:
                    raise AssertionError(f"quarter {k} never completed")

    nc.compile()
    return nc


_CACHE = {}


def _get_built(edge_index, edge_weight):
    x0 = np.zeros((N, C), np.float32)
    npad, sh, fwd, rev, rpw_o, rpw_i = _host_prep(x0, edge_index, edge_weight)
    nc = _build(npad, sh, fwd, rev)
    return npad, sh, fwd, rev, rpw_o, rpw_i, nc


def kernel(x, edge_index, edge_weight, w_z, b_z, w_r, b_r, w_h, b_h, lin_w, lin_b):
    import ml_dtypes
    from concourse.bass_utils import run_bass_kernel_spmd

    x = np.asarray(x, np.float32)
    edge_index = np.asarray(edge_index)
    edge_weight = np.asarray(edge_weight, np.float32)
    import hashlib
    key = hashlib.sha1(
        np.ascontiguousarray(edge_index).tobytes()
        + np.ascontiguousarray(edge_weight).tobytes()
    ).hexdigest()
    if key not in _CACHE:
        _CACHE.clear()
        _CACHE[key] = _get_built(edge_index, edge_weight)
    npad, sh, fwd, rev, rpw_o, rpw_i, nc = _CACHE[key]
    JW = sh // 512

    x_pad = np.zeros((npad, C), np.float32)
    x_pad[:N] = x
    xbf = x_pad.astype(ml_dtypes.bfloat16)

    wz = np.asarray(w_z, np.float32)
    wh = np.asarray(w_h, np.float32)

    def blk(d, k):
        b = np.zeros((C, 128), np.float32)
        b[:, 0:GATE] = wz[d, k, :C]
        b[:, GATE:128] = wh[d, k, :C]
        return b

    wsum4 = np.tile(blk(0, 0) + blk(1, 0), (4, 1))
    pairs = [(0, 1), (1, 1), (0, 2), (1, 2)]  # to1, ti1, to2, ti2 terms
    wblk4 = np.stack([np.tile(blk(d, k), (4, 1)) for d, k in pairs])
    bcat = np.concatenate(
        [np.asarray(b_z, np.float32), np.asarray(b_h, np.float32)]
    ).reshape(128, 1)
    nbneg = -bcat

    iota = np.tile(np.arange(128, dtype=np.float32), (128, 1))
    base = {
        "xbf": xbf,
        "io_bf": iota.astype(ml_dtypes.bfloat16),
        "idbf": np.eye(128, dtype=np.float32).astype(ml_dtypes.bfloat16),
        "wsum4": wsum4.astype(ml_dtypes.bfloat16),
        "wblk4": wblk4.astype(ml_dtypes.bfloat16),
        "bcat": bcat,
        "nbneg": nbneg,
        "linw": np.asarray(lin_w, np.float32).astype(ml_dtypes.bfloat16),
        "linb": np.asarray(lin_b, np.float32).reshape(OUTC, 1),
    }
    in_maps = []
    for c in range(NCORES):
        m = dict(base)
        m["xacc"] = np.ascontiguousarray(
            x_pad[c * sh:(c + 1) * sh]
            .reshape(JW, 4, 128, C).transpose(1, 3, 0, 2).reshape(128, JW * 128)
        ).astype(ml_dtypes.bfloat16)
        m["fidx"] = fwd["gidx"][c]
        m["fldst"] = fwd["ldst"][c]
        m["fwgt"] = fwd["wgt"][c]
        m["ridx"] = rev["gidx"][c]
        m["rldst"] = rev["ldst"][c]
        m["rwgt"] = rev["wgt"][c]
        m["rpwo"] = rpw_o[c]
        m["rpwi"] = rpw_i[c]
        in_maps.append(m)

    import os
    trace = bool(int(os.environ.get("DCRNN_TRACE", "0")))
    res = run_bass_kernel_spmd(
        nc, in_maps, core_ids=list(range(NCORES)), trace=trace
    )
    global LAST_EXEC_NS
    LAST_EXEC_NS = res.exec_time_ns
    out = np.concatenate([res.results[c]["outT"] for c in range(NCORES)], axis=1)
    return np.ascontiguousarray(out.T[:N]).astype(np.float32)


# revision 53
# speedup vs baseline: 2.4655x; 1.5114x over previous
"""DCRNN diffusion-conv GNN forward on 8 trn2 NeuronCores.

Math (reference has H0=0, so the r-gate is dead and every dconv input is x):
  rpo[v] = 1/deg_out[v]; rpi[v] = 1/deg_in[v]      (host, graph-cached)
  To1[d] = sum_{e:dst=d} rpo[src]*x[src]           (hop1 fwd, weighted one-hot)
  Ti1[s] = sum_{e:src=s} rpi[dst]*x[dst]           (hop1 rev)
  To2[d] = sum_{e:dst=d} (rpo*To1)[src]            (hop2 fwd, scale folded into
  Ti2[s] = sum_{e:src=s} (rpi*Ti1)[dst]             the all-gathered table)
  G_g = x@(Wg[0,0]+Wg[1,0])[:32] + To1@Wg[0,1][:32] + Ti1@Wg[1,1][:32]
        + To2@Wg[0,2][:32] + Ti2@Wg[1,2][:32] + b_g      for g in {z,h}
  out = relu(sigmoid(-G_z) * tanh(G_h)) @ lin_w + lin_b

Device structure: nodes sharded 8 ways; edges partitioned by scatter-side
shard. Per edge one 256B quad-row descriptor gathers the source row from a
replicated bf16 HBM table; a DVE-built one-hot (slot -> local dst row) turns
the segment sum into PE matmuls into a channel-major PSUM accumulator
acc[32*(w%4)+c, (w//4)*128+p] so the gate matmuls consume it directly with no
transposes. Cell budgets are maxed across cores (SPMD-homogeneous instruction
stream) but NOT rounded up to the 128-slot chunk grid: chunks that straddle a
cell boundary run one matmul per overlapped cell using variant ldst arrays
(-1 = not-mine slots). Hop1 results are rescaled and AllGathered as the hop2
tables, with each collective hidden under the next hop's gather stream.
"""

import sys

sys.path.insert(0, "/opt/trn_rl_repo")

import numpy as np

N = 100000
C = 32
NCORES = 8
GATE = 64
OUTC = 32
CALL = 1024  # slots per dma_gather (2048+ risks SWDGE ring deadlock, 4096 confirmed hangs)


def _wrap16(a):
    # dma_gather index layout: idx i lives at partition i%16, col i//16,
    # replicated to all eight 16-partition groups (shipped pre-replicated).
    s = a.shape[0]
    w = a.reshape(s // 16, 16).T.astype(np.int16)
    return np.ascontiguousarray(np.tile(w, (8, 1)))


def _prep_dir(gnode, snode, coef, npad, sh):
    """Slot/chunk structure for one propagate direction.

    gnode: gather-side node per edge (global), snode: scatter-side node
    (global, defines core = snode//sh), coef: per-edge hop1 weight.
    Returns per-core device arrays + core-independent chunk metadata.
    """
    W = sh // 128
    core = snode // sh
    nl = snode - core * sh
    w = nl // 128
    r = (nl % 128).astype(np.float32)
    q = gnode % 4
    ncell = W * 4
    cellid = w * 4 + q
    cnt = np.zeros((NCORES, ncell), np.int64)
    np.add.at(cnt, (core, cellid), 1)
    bud = cnt.max(axis=0)  # [ncell], NOT rounded: chunks may straddle cells
    starts = np.concatenate([[0], np.cumsum(bud)])
    S = int(starts[-1])
    S_pad = ((S + CALL - 1) // CALL) * CALL
    NV = S_pad // 128

    # ordinal of each cell within its first chunk (0 unless the chunk also
    # contains the tail of earlier cells / starts of smaller ones)
    nz0 = np.concatenate([[0], np.cumsum(bud > 0)])
    c128 = (starts[:-1] // 128) * 128
    t = np.searchsorted(starts, c128, side="right")
    lo = np.maximum(t - 1, 0)
    ocell = nz0[np.arange(ncell)] - nz0[lo]
    ocell[bud == 0] = 0
    V = int(ocell.max()) + 1

    # slot position of each edge: starts[cell] + rank within (core,cell)
    key = core * ncell + cellid
    order = np.argsort(key, kind="stable")
    ranks = np.empty(len(key), np.int64)
    sk = key[order]
    brk = np.concatenate([[0], np.nonzero(np.diff(sk))[0] + 1])
    grp = np.zeros(len(sk), np.int64)
    grp[brk] = brk
    grp = np.maximum.accumulate(grp)
    ranks[order] = np.arange(len(sk)) - grp
    pos = starts[cellid] + ranks
    var = np.where(pos // 128 == starts[cellid] // 128, ocell[cellid], 0)

    gidx = np.zeros((NCORES, S_pad), np.int64)
    ldst = np.full((NCORES, V, S_pad), -1.0, np.float32)
    wgt = np.zeros((NCORES, S_pad), np.float32)
    gidx[core, pos] = gnode // 4
    ldst[core, var, pos] = r
    wgt[core, pos] = coef

    # chunk segments (identical for every core): (chunk, variant, wi, qoff)
    segs = []
    for i in range(ncell):
        if bud[i] == 0:
            continue
        s, e = int(starts[i]), int(starts[i + 1])
        wi, qi = i // 4, i % 4
        for ch in range(s // 128, (e - 1) // 128 + 1):
            v = int(ocell[i]) if ch == s // 128 else 0
            segs.append((ch, v, wi, qi))
    segs.sort()
    first, last = {}, {}
    for idx, (ch, v, wi, qi) in enumerate(segs):
        if wi not in first:
            first[wi] = idx
        last[wi] = idx
    assert len(first) == W, "empty window: add dummy-segment support"
    chunks = [
        (ch, v, wi, qi * 32, idx == first[wi], idx == last[wi])
        for idx, (ch, v, wi, qi) in enumerate(segs)
    ]
    return dict(
        S=S_pad, NV=NV, V=V, chunks=chunks,
        gidx=[_wrap16(gidx[c]) for c in range(NCORES)],
        ldst=[np.ascontiguousarray(ldst[c].reshape(V, NV, 128).transpose(0, 2, 1))
              for c in range(NCORES)],
        wgt=[np.ascontiguousarray(wgt[c].reshape(NV, 128).T) for c in range(NCORES)],
    )


def _host_prep(x, edge_index, edge_weight):
    npad = ((N + 512 * NCORES - 1) // (512 * NCORES)) * 512 * NCORES
    sh = npad // NCORES
    src = edge_index[0].astype(np.int64)
    dst = edge_index[1].astype(np.int64)
    wv = edge_weight.astype(np.float64)
    deg_out = np.bincount(src, weights=wv, minlength=npad)
    deg_in = np.bincount(dst, weights=wv, minlength=npad)
    rpo = np.where(deg_out > 0, 1.0 / np.maximum(deg_out, 1e-300), 0.0).astype(np.float32)
    rpi = np.where(deg_in > 0, 1.0 / np.maximum(deg_in, 1e-300), 0.0).astype(np.float32)
    fwd = _prep_dir(src, dst, rpo[src], npad, sh)  # scatter by dst, gather src
    rev = _prep_dir(dst, src, rpi[dst], npad, sh)  # scatter by src, gather dst
    JW = sh // 512
    # drain rescale tables, p-major: rpw[p, j*4+q] = rp[core*sh + (4j+q)*128 + p]
    rpw_o = [np.ascontiguousarray(
        rpo[c * sh:(c + 1) * sh].reshape(JW, 4, 128).transpose(2, 0, 1).reshape(128, JW * 4))
        for c in range(NCORES)]
    rpw_i = [np.ascontiguousarray(
        rpi[c * sh:(c + 1) * sh].reshape(JW, 4, 128).transpose(2, 0, 1).reshape(128, JW * 4))
        for c in range(NCORES)]
    return npad, sh, fwd, rev, rpw_o, rpw_i


def _build(npad, sh, fwd, rev):
    import concourse.bacc as bacc
    import concourse.bass as bass
    import concourse.mybir as mybir
    import concourse.tile as tile

    W = sh // 128
    JW = W // 4
    AC = JW * 128  # accumulator free dim
    # j-chunks of the accumulator, drained as they complete; the tail is the
    # last chunk's gate work, so later chunks are finer
    JSPLIT = [JW - 18, 6, 6, 3, 3]
    J0 = [0] + list(np.cumsum(JSPLIT)[:-1])
    NSEG = len(JSPLIT)
    QCOL = [j * 128 for j in JSPLIT]
    f32 = mybir.dt.float32
    bf16 = mybir.dt.bfloat16
    i16 = mybir.dt.int16
    AF = mybir.ActivationFunctionType
    OP = mybir.AluOpType
    RG = [list(range(NCORES))]

    def quarter_of(j):
        for k in range(NSEG - 1, -1, -1):
            if j >= J0[k]:
                return k

    def tiles_of(jn):
        cols = jn * 128
        out = []
        c0 = 0
        while cols > 0:
            tw = 512 if cols >= 512 else cols
            out.append((c0, tw))
            c0 += tw
            cols -= tw
        return out

    nc = bacc.Bacc(target_bir_lowering=False)

    # ---------------- parameters ----------------
    xbf = nc.declare_dram_parameter("xbf", [npad, C], bf16, isOutput=False)
    xacc = nc.declare_dram_parameter("xacc", [128, AC], bf16, isOutput=False)
    io_bf = nc.declare_dram_parameter("io_bf", [128, 128], bf16, isOutput=False)
    idbf = nc.declare_dram_parameter("idbf", [128, 128], bf16, isOutput=False)
    wsum4 = nc.declare_dram_parameter("wsum4", [128, 128], bf16, isOutput=False)
    wblk4 = nc.declare_dram_parameter("wblk4", [4, 128, 128], bf16, isOutput=False)
    bcat = nc.declare_dram_parameter("bcat", [128, 1], f32, isOutput=False)
    nbneg = nc.declare_dram_parameter("nbneg", [128, 1], f32, isOutput=False)
    linw = nc.declare_dram_parameter("linw", [GATE, OUTC], bf16, isOutput=False)
    linb = nc.declare_dram_parameter("linb", [OUTC, 1], f32, isOutput=False)
    pin = {}
    for nm, d in (("f", fwd), ("r", rev)):
        pin[nm + "idx"] = nc.declare_dram_parameter(f"{nm}idx", [128, d["S"] // 16], i16, isOutput=False)
        pin[nm + "ldst"] = nc.declare_dram_parameter(f"{nm}ldst", [d["V"], 128, d["NV"]], f32, isOutput=False)
        pin[nm + "wgt"] = nc.declare_dram_parameter(f"{nm}wgt", [128, d["NV"]], f32, isOutput=False)
    rpwo = nc.declare_dram_parameter("rpwo", [128, JW * 4], f32, isOutput=False)
    rpwi = nc.declare_dram_parameter("rpwi", [128, JW * 4], f32, isOutput=False)
    outT = nc.declare_dram_parameter("outT", [OUTC, sh], f32, isOutput=True)

    TC = tile.TileContext

    def load_idx(pool, d, par, tag, chunks=1):
        isb = pool.tile([128, d["S"] // 16], i16, tag=tag, name=tag)
        n = d["S"] // 16
        step = -(-n // chunks)
        for c0 in range(0, n, step):
            c1 = min(c0 + step, n)
            nc.sync.dma_start(out=isb[:, c0:c1], in_=par[:, c0:c1])
        return isb

    def load_ldst(pool, d, par, tag):
        lsb = pool.tile([128, d["V"] * d["NV"]], f32, tag=tag, name=tag)
        for v in range(d["V"]):
            nc.sync.dma_start(
                out=lsb[:, v * d["NV"]:(v + 1) * d["NV"]], in_=par[v],
            )
        return lsb

    # -------- one gather+segment-sum pass (quarter-split accumulator) ------
    def hop_pass(pool, psum, d, isb, lsb, wsb, tab, iota, on_call_end=None):
        tabq = tab.rearrange("(g f) d -> g (f d)", f=4)
        accs = [
            psum.tile([128, QCOL[k]], f32, space="PSUM", tag="acc", name="acc")
            for k in range(NSEG)
        ]
        per = CALL // 128
        chmap = {}
        for e in d["chunks"]:
            chmap.setdefault(e[0] // per, []).append(e)
        NVd = d["NV"]
        for call in range(d["S"] // CALL):
            gt = pool.tile([128, per, 128], bf16, tag="gt", bufs=10, name="gt")
            nc.gpsimd.dma_gather(
                out_ap=gt[:],
                in_ap=tabq[:],
                idxs_ap=isb[:, call * (CALL // 16):(call + 1) * (CALL // 16)],
                num_idxs=CALL,
                num_idxs_reg=CALL,
                elem_size=128,
            )
            for ch, v, wi, qoff, st, sp in chmap.get(call, []):
                j = ch % per
                oh = pool.tile([128, 128], bf16, tag="oh", bufs=16, name="oh")
                lcol = lsb[:, v * NVd + ch: v * NVd + ch + 1]
                if wsb is None:
                    nc.vector.tensor_scalar(
                        out=oh[:], in0=iota[:], scalar1=lcol, scalar2=None,
                        op0=OP.is_equal,
                    )
                else:
                    nc.vector.tensor_scalar(
                        out=oh[:], in0=iota[:], scalar1=lcol,
                        scalar2=wsb[:, ch:ch + 1], op0=OP.is_equal, op1=OP.mult,
                    )
                wj = wi // 4
                k = quarter_of(wj)
                nc.tensor.matmul(
                    accs[k][32 * (wi % 4):32 * (wi % 4) + 32,
                            (wj - J0[k]) * 128:(wj - J0[k]) * 128 + 128],
                    lhsT=gt[:, j, qoff:qoff + 32],
                    rhs=oh[:],
                    start=st, stop=sp,
                    tile_position=(0, 32 * (wi % 4)),
                )
            if on_call_end is not None:
                on_call_end(call, accs)
        return accs

    # -------- drain one quarter: bf16 raster; rescaled node-major bounce ---
    def drain_q(cpool, psumt, acc_k, k, raster_slice, rp, sc, bounce, identbf):
        nc.vector.tensor_copy(out=raster_slice, in_=acc_k[:])
        if bounce is None:
            return
        j0, jn = J0[k], JSPLIT[k]
        for j in range(j0, j0 + jn):
            tp = psumt.tile([128, 128], bf16, space="PSUM", tag="dr_ps", name="dr_ps")
            nc.tensor.transpose(
                out=tp[:], in_=raster_slice[:, (j - j0) * 128:(j - j0 + 1) * 128],
                identity=identbf[:],
            )
            nc.vector.tensor_tensor(
                out=sc[:, j * 128:(j + 1) * 128].rearrange("p (q c) -> p q c", q=4),
                in0=tp[:].rearrange("p (q c) -> p q c", q=4),
                in1=rp[:, j * 4:(j + 1) * 4].rearrange("p (q o) -> p q o", o=1)
                    .broadcast_to([128, 4, C]),
                op=OP.mult,
            )
        nc.sync.dma_start(
            out=bounce.rearrange("(j q p) c -> p j q c", q=4, p=128)[:, j0:j0 + jn],
            in_=sc[:, j0 * 128:(j0 + jn) * 128]
                .rearrange("p (j q c) -> p j q c", j=jn, q=4),
        )

    # ================= single context =================
    with TC(nc) as tc:
        with (
            tc.tile_pool(name="p", bufs=3) as pool,
            tc.tile_pool(name="c", bufs=1) as cpool,
            tc.tile_pool(name="w", bufs=2) as wpool,
            tc.tile_pool(name="d", bufs=1, space="DRAM") as dpool,
            tc.tile_pool(name="ps", bufs=2, space="PSUM") as psum,
            tc.tile_pool(name="pst", bufs=1, space="PSUM") as psumt,
            tc.tile_pool(name="psg", bufs=1, space="PSUM") as psg,
        ):
            t1o_b = dpool.tile([sh, C], bf16, name="t1o_b")
            t1i_b = dpool.tile([sh, C], bf16, name="t1i_b")
            t1o_tab = dpool.tile([npad, C], bf16, name="t1o_tab")
            t1i_tab = dpool.tile([npad, C], bf16, name="t1i_tab")

            iota = cpool.tile([128, 128], bf16, name="iota")
            identbf = cpool.tile([128, 128], bf16, name="identbf")
            # critical-path loads first: pass-1 gathers need only iota+isb_f
            nc.sync.dma_start(out=iota[:], in_=io_bf[:])
            isb_f = load_idx(cpool, fwd, pin["fidx"], "isb_f")
            lsb_f = load_ldst(cpool, fwd, pin["fldst"], "lsb_f")
            wsb_f = wpool.tile([128, fwd["NV"]], f32, tag="wgt", name="wsb_f")
            nc.sync.dma_start(out=wsb_f[:], in_=pin["fwgt"][:])
            nc.sync.dma_start(out=identbf[:], in_=idbf[:])

            to1_sb = cpool.tile([128, AC], bf16, name="to1_sb")
            ti1_sb = cpool.tile([128, AC], bf16, name="ti1_sb")
            to2_sb = cpool.tile([128, AC], bf16, name="to2_sb")
            ti2_k = [
                cpool.tile([128, QCOL[k]], bf16, tag=f"ti2_{k}", name=f"ti2_{k}")
                for k in range(NSEG)
            ]
            xacc_sb = cpool.tile([128, AC], bf16, name="xacc_sb")
            ws = cpool.tile([128, 128], bf16, name="ws")
            wb = cpool.tile([128, 4, 128], bf16, name="wb")
            nb = cpool.tile([128, 1], f32, name="nb")
            ngb = cpool.tile([128, 1], f32, name="ngb")
            lw = cpool.tile([GATE, OUTC], bf16, name="lw")
            lb = cpool.tile([OUTC, 1], f32, name="lb")
            rpo_sb = cpool.tile([128, JW * 4], f32, name="rpo_sb")
            rpi_sb = cpool.tile([128, JW * 4], f32, name="rpi_sb")
            sc = cpool.tile([128, AC], bf16, tag="dr_sc", name="dr_sc")

            # ---------------- pass 1: hop1 fwd ----------------
            accs = hop_pass(pool, psum, fwd, isb_f, lsb_f, wsb_f, xbf, iota)
            # off-critical-path loads overlap the pass-1 gather stream
            isb_r = load_idx(cpool, rev, pin["ridx"], "isb_r", chunks=6)
            lsb_r = load_ldst(cpool, rev, pin["rldst"], "lsb_r")
            wsb_r = wpool.tile([128, rev["NV"]], f32, tag="wgt", name="wsb_r")
            nc.sync.dma_start(out=wsb_r[:], in_=pin["rwgt"][:])
            nc.sync.dma_start(out=xacc_sb[:], in_=xacc[:])
            nc.sync.dma_start(out=ws[:], in_=wsum4[:])
            nc.sync.dma_start(out=wb[:], in_=wblk4.rearrange("r p n -> p r n"))
            nc.sync.dma_start(out=nb[:], in_=bcat[:])
            nc.sync.dma_start(out=ngb[:], in_=nbneg[:])
            nc.sync.dma_start(out=lw[:], in_=linw[:])
            nc.sync.dma_start(out=lb[:], in_=linb[:])
            nc.sync.dma_start(out=rpo_sb[:], in_=rpwo[:])
            nc.sync.dma_start(out=rpi_sb[:], in_=rpwi[:])

            for k in range(NSEG):
                drain_q(cpool, psumt, accs[k], k,
                        to1_sb[:, J0[k] * 128:(J0[k] + JSPLIT[k]) * 128],
                        rpo_sb, sc, t1o_b, identbf)
            nc.gpsimd.collective_compute(
                "AllGather", OP.bypass, replica_groups=RG,
                ins=[t1o_b.opt()], outs=[t1o_tab.opt()],
            )

            # ---------------- pass 2: hop1 rev ----------------
            accs = hop_pass(pool, psum, rev, isb_r, lsb_r, wsb_r, xbf, iota)
            for k in range(NSEG):
                drain_q(cpool, psumt, accs[k], k,
                        ti1_sb[:, J0[k] * 128:(J0[k] + JSPLIT[k]) * 128],
                        rpi_sb, sc, t1i_b, identbf)
            nc.gpsimd.collective_compute(
                "AllGather", OP.bypass, replica_groups=RG,
                ins=[t1i_b.opt()], outs=[t1i_tab.opt()],
            )

            # ---------------- pass 3: hop2 fwd ----------------
            accs = hop_pass(pool, psum, fwd, isb_f, lsb_f, None, t1o_tab, iota)
            for k in range(NSEG):
                drain_q(cpool, psumt, accs[k], k,
                        to2_sb[:, J0[k] * 128:(J0[k] + JSPLIT[k]) * 128],
                        None, None, None, identbf)

            # ---------------- pass 4: hop2 rev + gates -------------------
            # quarter drains land in ti2_k[k]; gate tiles for quarter k start
            # as soon as it drains, hiding most of the gate phase under the
            # hop2r gather stream.
            outv = outT.rearrange("c (j q p) -> q c j p", q=4, p=128)
            per = CALL // 128
            qlast = {}
            for e in rev["chunks"]:
                kq = quarter_of(e[2] // 4)
                qlast[kq] = max(qlast.get(kq, -1), e[0] // per)
            ncalls_r = rev["S"] // CALL

            def emit_quarter(k, accs):
                nc.vector.tensor_copy(out=ti2_k[k][:], in_=accs[k][:])
                j0, jn = J0[k], JSPLIT[k]
                for qi in range(4):
                    sl4 = slice(32 * qi, 32 * qi + 32)
                    ot = pool.tile([OUTC, max(QCOL)], f32, tag="g_ot", bufs=2, name="ot")
                    for c0, TW in tiles_of(jn):
                        gsl = slice(j0 * 128 + c0, j0 * 128 + c0 + TW)
                        G = psg.tile([128, 512], f32, space="PSUM", tag="G",
                                     bufs=2, name="G")
                        terms = [
                            (ws[sl4, :], xacc_sb[sl4, gsl]),
                            (wb[sl4, 0, :], to1_sb[sl4, gsl]),
                            (wb[sl4, 1, :], ti1_sb[sl4, gsl]),
                            (wb[sl4, 2, :], to2_sb[sl4, gsl]),
                            (wb[sl4, 3, :], ti2_k[k][sl4, c0:c0 + TW]),
                        ]
                        for ti, (lhsT, rhs) in enumerate(terms):
                            nc.tensor.matmul(
                                G[:, 0:TW], lhsT=lhsT, rhs=rhs,
                                start=(ti == 0), stop=(ti == len(terms) - 1),
                                tile_position=(32 * qi, 0),
                            )
                        zb = pool.tile([GATE, 512], bf16, tag="g_zb", name="zb")
                        ht = pool.tile([GATE, 512], bf16, tag="g_ht", name="ht")
                        nc.scalar.activation(
                            out=zb[:, 0:TW], in_=G[0:GATE, 0:TW], func=AF.Sigmoid,
                            bias=ngb[0:GATE, :], scale=-1.0,
                        )
                        nc.scalar.activation(
                            out=ht[:, 0:TW], in_=G[GATE:128, 0:TW], func=AF.Tanh,
                            bias=nb[GATE:128, :], scale=1.0,
                        )
                        hs = pool.tile([GATE, 512], bf16, tag="g_hs", name="hs")
                        nc.vector.tensor_tensor(
                            out=hs[:, 0:TW], in0=zb[:, 0:TW], in1=ht[:, 0:TW],
                            op=OP.mult,
                        )
                        hr = pool.tile([GATE, 512], bf16, tag="g_hr", name="hr")
                        nc.vector.tensor_scalar(
                            out=hr[:, 0:TW], in0=hs[:, 0:TW], scalar1=0.0,
                            scalar2=None, op0=OP.max,
                        )
                        po = psg.tile([OUTC, 512], f32, space="PSUM", tag="po",
                                      bufs=1, name="po")
                        nc.tensor.matmul(
                            po[:, 0:TW], lhsT=lw[:], rhs=hr[:, 0:TW],
                            start=True, stop=True,
                        )
                        nc.vector.tensor_scalar(
                            out=ot[:, c0:c0 + TW], in0=po[:, 0:TW], scalar1=lb[:],
                            scalar2=None, op0=OP.add,
                        )
                    nc.sync.dma_start(
                        out=outv[qi][:, j0:j0 + jn, :],
                        in_=ot[:, 0:jn * 128].rearrange("c (j p) -> c j p", p=128),
                    )

            done = set()

            def on_call_end(call, accs):
                for k in range(NSEG):
                    if qlast.get(k) == call and k not in done:
                        done.add(k)
                        emit_quarter(k, accs)

            hop_pass(pool, psum, rev, isb_r, lsb_r, None, t1i_tab, iota,
                     on_call_end=on_call_end)
            for k in range(NSEG):
                if k not in done:
                    raise AssertionError(f"quarter {k} never completed")

    nc.compile()
    return nc


_CACHE = {}
_JIT = {}


def _get_built(edge_index, edge_weight):
    x0 = np.zeros((N, C), np.float32)
    npad, sh, fwd, rev, rpw_o, rpw_i = _host_prep(x0, edge_index, edge_weight)
    nc = _build(npad, sh, fwd, rev)
    return npad, sh, fwd, rev, rpw_o, rpw_i, nc


def _run_spmd_cached(nc, in_maps):
    """run_bass_kernel_spmd's axon path with the jitted executable cached
    across calls (the library re-jits a fresh closure per call, costing ~2s
    of retrace + BIR verify on every invocation)."""
    import jax
    import numpy as _np
    from jax.sharding import Mesh, PartitionSpec
    from jax.experimental.shard_map import shard_map
    import concourse.mybir as mybir
    from concourse import bass2jax

    n_cores = NCORES
    key = id(nc)
    if key not in _JIT:
        bass2jax.install_neuronx_cc_hook()
        pname = nc.partition_id_tensor.name if nc.partition_id_tensor else None
        in_names, out_names, out_avals, zero_shapes = [], [], [], []
        for alloc in nc.m.functions[0].allocations:
            if not isinstance(alloc, mybir.MemoryLocationSet):
                continue
            name = alloc.memorylocations[0].name
            if alloc.kind == "ExternalInput":
                if name != pname:
                    in_names.append(name)
            elif alloc.kind == "ExternalOutput":
                out_names.append(name)
                shape = tuple(alloc.tensor_shape)
                dtype = mybir.dt.np(alloc.dtype)
                out_avals.append(jax.core.ShapedArray(shape, dtype))
                zero_shapes.append((shape, dtype))
        n_params = len(in_names)
        all_names = in_names + out_names
        if pname is not None:
            all_names = all_names + [pname]

        def _body(*args):
            operands = list(args)
            if pname is not None:
                operands.append(bass2jax.partition_id_tensor())
            outs = bass2jax._bass_exec_p.bind(
                *operands,
                out_avals=tuple(out_avals),
                in_names=tuple(all_names),
                out_names=tuple(out_names),
                lowering_input_output_aliases=(),
                sim_require_finite=True,
                sim_require_nnan=True,
                nc=nc,
            )
            return tuple(outs)

        devices = jax.devices()[:n_cores]
        mesh = Mesh(_np.asarray(devices), ("core",))
        n_outs = len(out_names)
        sharded = jax.jit(
            shard_map(
                _body, mesh=mesh,
                in_specs=(PartitionSpec("core"),) * (n_params + n_outs),
                out_specs=(PartitionSpec("core"),) * n_outs,
                check_rep=False,
            ),
            donate_argnums=tuple(range(n_params, n_params + n_outs)),
            keep_unused=True,
        )
        _JIT.clear()
        _JIT[key] = (sharded, in_names, out_names, out_avals, zero_shapes)
    sharded, in_names, out_names, out_avals, zero_shapes = _JIT[key]

    concat_in = [
        np.concatenate([np.asarray(m[name]) for m in in_maps], axis=0)
        for name in in_names
    ]
    concat_zeros = [
        np.zeros((n_cores * s[0], *s[1:]), d) for s, d in zero_shapes
    ]
    out_arrs = sharded(*concat_in, *concat_zeros)
    return [
        {
            name: np.asarray(out_arrs[i]).reshape(n_cores, *out_avals[i].shape)[c]
            for i, name in enumerate(out_names)
        }
        for c in range(n_cores)
    ]


def kernel(x, edge_index, edge_weight, w_z, b_z, w_r, b_r, w_h, b_h, lin_w, lin_b):
    import ml_dtypes
    from concourse.bass_utils import run_bass_kernel_spmd

    x = np.asarray(x, np.float32)
    edge_index = np.asarray(edge_index)
    edge_weight = np.asarray(edge_weight, np.float32)
    import hashlib
    key = hashlib.sha1(
        np.ascontiguousarray(edge_index).tobytes()
        + np.ascontiguousarray(edge_weight).tobytes()
    ).hexdigest()
    if key not in _CACHE:
        _CACHE.clear()
        _CACHE[key] = _get_built(edge_index, edge_weight)
    npad, sh, fwd, rev, rpw_o, rpw_i, nc = _CACHE[key]
    JW = sh // 512

    x_pad = np.zeros((npad, C), np.float32)
    x_pad[:N] = x
    xbf = x_pad.astype(ml_dtypes.bfloat16)

    wz = np.asarray(w_z, np.float32)
    wh = np.asarray(w_h, np.float32)

    def blk(d, k):
        b = np.zeros((C, 128), np.float32)
        b[:, 0:GATE] = wz[d, k, :C]
        b[:, GATE:128] = wh[d, k, :C]
        return b

    wsum4 = np.tile(blk(0, 0) + blk(1, 0), (4, 1))
    pairs = [(0, 1), (1, 1), (0, 2), (1, 2)]  # to1, ti1, to2, ti2 terms
    wblk4 = np.stack([np.tile(blk(d, k), (4, 1)) for d, k in pairs])
    bcat = np.concatenate(
        [np.asarray(b_z, np.float32), np.asarray(b_h, np.float32)]
    ).reshape(128, 1)
    nbneg = -bcat

    iota = np.tile(np.arange(128, dtype=np.float32), (128, 1))
    base = {
        "xbf": xbf,
        "io_bf": iota.astype(ml_dtypes.bfloat16),
        "idbf": np.eye(128, dtype=np.float32).astype(ml_dtypes.bfloat16),
        "wsum4": wsum4.astype(ml_dtypes.bfloat16),
        "wblk4": wblk4.astype(ml_dtypes.bfloat16),
        "bcat": bcat,
        "nbneg": nbneg,
        "linw": np.asarray(lin_w, np.float32).astype(ml_dtypes.bfloat16),
        "linb": np.asarray(lin_b, np.float32).reshape(OUTC, 1),
    }
    in_maps = []
    for c in range(NCORES):
        m = dict(base)
        m["xacc"] = np.ascontiguousarray(
            x_pad[c * sh:(c + 1) * sh]
            .reshape(JW, 4, 128, C).transpose(1, 3, 0, 2).reshape(128, JW * 128)
        ).astype(ml_dtypes.bfloat16)
        m["fidx"] = fwd["gidx"][c]
        m["fldst"] = fwd["ldst"][c]
        m["fwgt"] = fwd["wgt"][c]
        m["ridx"] = rev["gidx"][c]
        m["rldst"] = rev["ldst"][c]
        m["rwgt"] = rev["wgt"][c]
        m["rpwo"] = rpw_o[c]
        m["rpwi"] = rpw_i[c]
        in_maps.append(m)

    import os
    trace = bool(int(os.environ.get("DCRNN_TRACE", "0")))
    global LAST_EXEC_NS
    if trace:
        res = run_bass_kernel_spmd(
            nc, in_maps, core_ids=list(range(NCORES)), trace=True
        )
        LAST_EXEC_NS = res.exec_time_ns
        results = res.results
    else:
        LAST_EXEC_NS = None
        results = _run_spmd_cached(nc, in_maps)
    out = np.concatenate([results[c]["outT"] for c in range(NCORES)], axis=1)
    return np.ascontiguousarray(out.T[:N]).astype(np.float32)


# revision 54
# speedup vs baseline: 5.2040x; 2.1108x over previous
"""DCRNN diffusion-conv GNN forward on 8 trn2 NeuronCores.

Math (reference has H0=0, so the r-gate is dead and every dconv input is x):
  rpo[v] = 1/deg_out[v]; rpi[v] = 1/deg_in[v]      (host, graph-cached)
  To1[d] = sum_{e:dst=d} rpo[src]*x[src]           (hop1 fwd, weighted one-hot)
  Ti1[s] = sum_{e:src=s} rpi[dst]*x[dst]           (hop1 rev)
  To2[d] = sum_{e:dst=d} (rpo*To1)[src]            (hop2 fwd, scale folded into
  Ti2[s] = sum_{e:src=s} (rpi*Ti1)[dst]             the all-gathered table)
  G_g = x@(Wg[0,0]+Wg[1,0])[:32] + To1@Wg[0,1][:32] + Ti1@Wg[1,1][:32]
        + To2@Wg[0,2][:32] + Ti2@Wg[1,2][:32] + b_g      for g in {z,h}
  out = relu(sigmoid(-G_z) * tanh(G_h)) @ lin_w + lin_b

Device structure: nodes sharded 8 ways; edges partitioned by scatter-side
shard. Per edge one 256B quad-row descriptor gathers the source row from a
replicated bf16 HBM table; a DVE-built one-hot (slot -> local dst row) turns
the segment sum into PE matmuls into a channel-major PSUM accumulator
acc[32*(w%4)+c, (w//4)*128+p] so the gate matmuls consume it directly with no
transposes. Cell budgets are maxed across cores (SPMD-homogeneous instruction
stream) but NOT rounded up to the 128-slot chunk grid: chunks that straddle a
cell boundary run one matmul per overlapped cell using variant ldst arrays
(-1 = not-mine slots). Hop1 results are rescaled and AllGathered as the hop2
tables, with each collective hidden under the next hop's gather stream.
"""

import sys

sys.path.insert(0, "/opt/trn_rl_repo")

import numpy as np

N = 100000
C = 32
NCORES = 8
GATE = 64
OUTC = 32
CALL = 1024  # slots per dma_gather (2048+ risks SWDGE ring deadlock, 4096 confirmed hangs)


def _wrap16(a):
    # dma_gather index layout: idx i lives at partition i%16, col i//16,
    # replicated to all eight 16-partition groups (shipped pre-replicated).
    s = a.shape[0]
    w = a.reshape(s // 16, 16).T.astype(np.int16)
    return np.ascontiguousarray(np.tile(w, (8, 1)))


def _prep_dir(gnode, snode, coef, npad, sh):
    """Slot/chunk structure for one propagate direction.

    gnode: gather-side node per edge (global), snode: scatter-side node
    (global, defines core = snode//sh), coef: per-edge hop1 weight.
    Returns per-core device arrays + core-independent chunk metadata.
    """
    W = sh // 128
    core = snode // sh
    nl = snode - core * sh
    w = nl // 128
    r = (nl % 128).astype(np.float32)
    q = gnode % 4
    ncell = W * 4
    cellid = w * 4 + q
    cnt = np.zeros((NCORES, ncell), np.int64)
    np.add.at(cnt, (core, cellid), 1)
    bud = cnt.max(axis=0)  # [ncell], NOT rounded: chunks may straddle cells
    starts = np.concatenate([[0], np.cumsum(bud)])
    S = int(starts[-1])
    S_pad = ((S + CALL - 1) // CALL) * CALL
    NV = S_pad // 128

    # ordinal of each cell within its first chunk (0 unless the chunk also
    # contains the tail of earlier cells / starts of smaller ones)
    nz0 = np.concatenate([[0], np.cumsum(bud > 0)])
    c128 = (starts[:-1] // 128) * 128
    t = np.searchsorted(starts, c128, side="right")
    lo = np.maximum(t - 1, 0)
    ocell = nz0[np.arange(ncell)] - nz0[lo]
    ocell[bud == 0] = 0
    V = int(ocell.max()) + 1

    # slot position of each edge: starts[cell] + rank within (core,cell)
    key = core * ncell + cellid
    order = np.argsort(key, kind="stable")
    ranks = np.empty(len(key), np.int64)
    sk = key[order]
    brk = np.concatenate([[0], np.nonzero(np.diff(sk))[0] + 1])
    grp = np.zeros(len(sk), np.int64)
    grp[brk] = brk
    grp = np.maximum.accumulate(grp)
    ranks[order] = np.arange(len(sk)) - grp
    pos = starts[cellid] + ranks
    var = np.where(pos // 128 == starts[cellid] // 128, ocell[cellid], 0)

    gidx = np.zeros((NCORES, S_pad), np.int64)
    ldst = np.full((NCORES, V, S_pad), -1.0, np.float32)
    wgt = np.zeros((NCORES, S_pad), np.float32)
    gidx[core, pos] = gnode // 4
    ldst[core, var, pos] = r
    wgt[core, pos] = coef

    # chunk segments (identical for every core): (chunk, variant, wi, qoff)
    segs = []
    for i in range(ncell):
        if bud[i] == 0:
            continue
        s, e = int(starts[i]), int(starts[i + 1])
        wi, qi = i // 4, i % 4
        for ch in range(s // 128, (e - 1) // 128 + 1):
            v = int(ocell[i]) if ch == s // 128 else 0
            segs.append((ch, v, wi, qi))
    segs.sort()
    first, last = {}, {}
    for idx, (ch, v, wi, qi) in enumerate(segs):
        if wi not in first:
            first[wi] = idx
        last[wi] = idx
    assert len(first) == W, "empty window: add dummy-segment support"
    chunks = [
        (ch, v, wi, qi * 32, idx == first[wi], idx == last[wi])
        for idx, (ch, v, wi, qi) in enumerate(segs)
    ]
    return dict(
        S=S_pad, NV=NV, V=V, chunks=chunks,
        gidx=[_wrap16(gidx[c]) for c in range(NCORES)],
        ldst=[np.ascontiguousarray(ldst[c].reshape(V, NV, 128).transpose(0, 2, 1))
              for c in range(NCORES)],
        wgt=[np.ascontiguousarray(wgt[c].reshape(NV, 128).T) for c in range(NCORES)],
    )


def _host_prep(x, edge_index, edge_weight):
    npad = ((N + 512 * NCORES - 1) // (512 * NCORES)) * 512 * NCORES
    sh = npad // NCORES
    src = edge_index[0].astype(np.int64)
    dst = edge_index[1].astype(np.int64)
    wv = edge_weight.astype(np.float64)
    deg_out = np.bincount(src, weights=wv, minlength=npad)
    deg_in = np.bincount(dst, weights=wv, minlength=npad)
    rpo = np.where(deg_out > 0, 1.0 / np.maximum(deg_out, 1e-300), 0.0).astype(np.float32)
    rpi = np.where(deg_in > 0, 1.0 / np.maximum(deg_in, 1e-300), 0.0).astype(np.float32)
    fwd = _prep_dir(src, dst, rpo[src], npad, sh)  # scatter by dst, gather src
    rev = _prep_dir(dst, src, rpi[dst], npad, sh)  # scatter by src, gather dst
    JW = sh // 512
    # drain rescale tables, p-major: rpw[p, j*4+q] = rp[core*sh + (4j+q)*128 + p]
    rpw_o = [np.ascontiguousarray(
        rpo[c * sh:(c + 1) * sh].reshape(JW, 4, 128).transpose(2, 0, 1).reshape(128, JW * 4))
        for c in range(NCORES)]
    rpw_i = [np.ascontiguousarray(
        rpi[c * sh:(c + 1) * sh].reshape(JW, 4, 128).transpose(2, 0, 1).reshape(128, JW * 4))
        for c in range(NCORES)]
    return npad, sh, fwd, rev, rpw_o, rpw_i


def _build(npad, sh, fwd, rev):
    import concourse.bacc as bacc
    import concourse.bass as bass
    import concourse.mybir as mybir
    import concourse.tile as tile

    W = sh // 128
    JW = W // 4
    AC = JW * 128  # accumulator free dim
    # j-chunks of the accumulator, drained as they complete; the tail is the
    # last chunk's gate work, so later chunks are finer
    JSPLIT = [JW - 18, 6, 6, 3, 3]
    J0 = [0] + list(np.cumsum(JSPLIT)[:-1])
    NSEG = len(JSPLIT)
    QCOL = [j * 128 for j in JSPLIT]
    f32 = mybir.dt.float32
    bf16 = mybir.dt.bfloat16
    i16 = mybir.dt.int16
    AF = mybir.ActivationFunctionType
    OP = mybir.AluOpType
    RG = [list(range(NCORES))]

    def quarter_of(j):
        for k in range(NSEG - 1, -1, -1):
            if j >= J0[k]:
                return k

    def tiles_of(jn):
        cols = jn * 128
        out = []
        c0 = 0
        while cols > 0:
            tw = 512 if cols >= 512 else cols
            out.append((c0, tw))
            c0 += tw
            cols -= tw
        return out

    nc = bacc.Bacc(target_bir_lowering=False)

    # ---------------- parameters ----------------
    xbf = nc.declare_dram_parameter("xbf", [npad, C], bf16, isOutput=False)
    xacc = nc.declare_dram_parameter("xacc", [128, AC], bf16, isOutput=False)
    io_bf = nc.declare_dram_parameter("io_bf", [128, 128], bf16, isOutput=False)
    idbf = nc.declare_dram_parameter("idbf", [128, 128], bf16, isOutput=False)
    wsum4 = nc.declare_dram_parameter("wsum4", [128, 128], bf16, isOutput=False)
    wblk4 = nc.declare_dram_parameter("wblk4", [4, 128, 128], bf16, isOutput=False)
    bcat = nc.declare_dram_parameter("bcat", [128, 1], f32, isOutput=False)
    nbneg = nc.declare_dram_parameter("nbneg", [128, 1], f32, isOutput=False)
    linw = nc.declare_dram_parameter("linw", [GATE, OUTC], bf16, isOutput=False)
    linb = nc.declare_dram_parameter("linb", [OUTC, 1], f32, isOutput=False)
    pin = {}
    for nm, d in (("f", fwd), ("r", rev)):
        pin[nm + "idx"] = nc.declare_dram_parameter(f"{nm}idx", [128, d["S"] // 16], i16, isOutput=False)
        pin[nm + "ldst"] = nc.declare_dram_parameter(f"{nm}ldst", [d["V"], 128, d["NV"]], f32, isOutput=False)
        pin[nm + "wgt"] = nc.declare_dram_parameter(f"{nm}wgt", [128, d["NV"]], f32, isOutput=False)
    rpwo = nc.declare_dram_parameter("rpwo", [128, JW * 4], f32, isOutput=False)
    rpwi = nc.declare_dram_parameter("rpwi", [128, JW * 4], f32, isOutput=False)
    outT = nc.declare_dram_parameter("outT", [OUTC, sh], f32, isOutput=True)

    TC = tile.TileContext

    def load_idx(pool, d, par, tag, chunks=1):
        isb = pool.tile([128, d["S"] // 16], i16, tag=tag, name=tag)
        n = d["S"] // 16
        step = -(-n // chunks)
        for c0 in range(0, n, step):
            c1 = min(c0 + step, n)
            nc.sync.dma_start(out=isb[:, c0:c1], in_=par[:, c0:c1])
        return isb

    def load_ldst(pool, d, par, tag):
        lsb = pool.tile([128, d["V"] * d["NV"]], f32, tag=tag, name=tag)
        for v in range(d["V"]):
            nc.sync.dma_start(
                out=lsb[:, v * d["NV"]:(v + 1) * d["NV"]], in_=par[v],
            )
        return lsb

    # -------- one gather+segment-sum pass (quarter-split accumulator) ------
    def hop_pass(pool, psum, d, isb, lsb, wsb, tab, iota, on_call_end=None):
        tabq = tab.rearrange("(g f) d -> g (f d)", f=4)
        accs = [
            psum.tile([128, QCOL[k]], f32, space="PSUM", tag="acc", name="acc")
            for k in range(NSEG)
        ]
        per = CALL // 128
        chmap = {}
        for e in d["chunks"]:
            chmap.setdefault(e[0] // per, []).append(e)
        NVd = d["NV"]
        for call in range(d["S"] // CALL):
            gt = pool.tile([128, per, 128], bf16, tag="gt", bufs=10, name="gt")
            nc.gpsimd.dma_gather(
                out_ap=gt[:],
                in_ap=tabq[:],
                idxs_ap=isb[:, call * (CALL // 16):(call + 1) * (CALL // 16)],
                num_idxs=CALL,
                num_idxs_reg=CALL,
                elem_size=128,
            )
            for ch, v, wi, qoff, st, sp in chmap.get(call, []):
                j = ch % per
                oh = pool.tile([128, 128], bf16, tag="oh", bufs=16, name="oh")
                lcol = lsb[:, v * NVd + ch: v * NVd + ch + 1]
                if wsb is None:
                    nc.vector.tensor_scalar(
                        out=oh[:], in0=iota[:], scalar1=lcol, scalar2=None,
                        op0=OP.is_equal,
                    )
                else:
                    nc.vector.tensor_scalar(
                        out=oh[:], in0=iota[:], scalar1=lcol,
                        scalar2=wsb[:, ch:ch + 1], op0=OP.is_equal, op1=OP.mult,
                    )
                wj = wi // 4
                k = quarter_of(wj)
                nc.tensor.matmul(
                    accs[k][32 * (wi % 4):32 * (wi % 4) + 32,
                            (wj - J0[k]) * 128:(wj - J0[k]) * 128 + 128],
                    lhsT=gt[:, j, qoff:qoff + 32],
                    rhs=oh[:],
                    start=st, stop=sp,
                    tile_position=(0, 32 * (wi % 4)),
                )
            if on_call_end is not None:
                on_call_end(call, accs)
        return accs

    # -------- drain one quarter: bf16 raster; rescaled node-major bounce ---
    def drain_q(cpool, psumt, acc_k, k, raster_slice, rp, sc, bounce, identbf):
        nc.vector.tensor_copy(out=raster_slice, in_=acc_k[:])
        if bounce is None:
            return
        j0, jn = J0[k], JSPLIT[k]
        for j in range(j0, j0 + jn):
            tp = psumt.tile([128, 128], bf16, space="PSUM", tag="dr_ps", name="dr_ps")
            nc.tensor.transpose(
                out=tp[:], in_=raster_slice[:, (j - j0) * 128:(j - j0 + 1) * 128],
                identity=identbf[:],
            )
            nc.vector.tensor_tensor(
                out=sc[:, j * 128:(j + 1) * 128].rearrange("p (q c) -> p q c", q=4),
                in0=tp[:].rearrange("p (q c) -> p q c", q=4),
                in1=rp[:, j * 4:(j + 1) * 4].rearrange("p (q o) -> p q o", o=1)
                    .broadcast_to([128, 4, C]),
                op=OP.mult,
            )
        nc.sync.dma_start(
            out=bounce.rearrange("(j q p) c -> p j q c", q=4, p=128)[:, j0:j0 + jn],
            in_=sc[:, j0 * 128:(j0 + jn) * 128]
                .rearrange("p (j q c) -> p j q c", j=jn, q=4),
        )

    # ================= single context =================
    with TC(nc) as tc:
        with (
            tc.tile_pool(name="p", bufs=3) as pool,
            tc.tile_pool(name="c", bufs=1) as cpool,
            tc.tile_pool(name="w", bufs=2) as wpool,
            tc.tile_pool(name="d", bufs=1, space="DRAM") as dpool,
            tc.tile_pool(name="ps", bufs=2, space="PSUM") as psum,
            tc.tile_pool(name="pst", bufs=1, space="PSUM") as psumt,
            tc.tile_pool(name="psg", bufs=1, space="PSUM") as psg,
        ):
            t1o_b = dpool.tile([sh, C], bf16, name="t1o_b")
            t1i_b = dpool.tile([sh, C], bf16, name="t1i_b")
            t1o_tab = dpool.tile([npad, C], bf16, name="t1o_tab")
            t1i_tab = dpool.tile([npad, C], bf16, name="t1i_tab")

            iota = cpool.tile([128, 128], bf16, name="iota")
            identbf = cpool.tile([128, 128], bf16, name="identbf")
            # critical-path loads first: pass-1 gathers need only iota+isb_f
            nc.sync.dma_start(out=iota[:], in_=io_bf[:])
            isb_f = load_idx(cpool, fwd, pin["fidx"], "isb_f")
            lsb_f = load_ldst(cpool, fwd, pin["fldst"], "lsb_f")
            wsb_f = wpool.tile([128, fwd["NV"]], f32, tag="wgt", name="wsb_f")
            nc.sync.dma_start(out=wsb_f[:], in_=pin["fwgt"][:])
            nc.sync.dma_start(out=identbf[:], in_=idbf[:])

            to1_sb = cpool.tile([128, AC], bf16, name="to1_sb")
            ti1_sb = cpool.tile([128, AC], bf16, name="ti1_sb")
            to2_sb = cpool.tile([128, AC], bf16, name="to2_sb")
            ti2_k = [
                cpool.tile([128, QCOL[k]], bf16, tag=f"ti2_{k}", name=f"ti2_{k}")
                for k in range(NSEG)
            ]
            xacc_sb = cpool.tile([128, AC], bf16, name="xacc_sb")
            ws = cpool.tile([128, 128], bf16, name="ws")
            wb = cpool.tile([128, 4, 128], bf16, name="wb")
            nb = cpool.tile([128, 1], f32, name="nb")
            ngb = cpool.tile([128, 1], f32, name="ngb")
            lw = cpool.tile([GATE, OUTC], bf16, name="lw")
            lb = cpool.tile([OUTC, 1], f32, name="lb")
            rpo_sb = cpool.tile([128, JW * 4], f32, name="rpo_sb")
            rpi_sb = cpool.tile([128, JW * 4], f32, name="rpi_sb")
            sc = cpool.tile([128, AC], bf16, tag="dr_sc", name="dr_sc")

            # ---------------- pass 1: hop1 fwd ----------------
            accs = hop_pass(pool, psum, fwd, isb_f, lsb_f, wsb_f, xbf, iota)
            # off-critical-path loads overlap the pass-1 gather stream
            isb_r = load_idx(cpool, rev, pin["ridx"], "isb_r", chunks=6)
            lsb_r = load_ldst(cpool, rev, pin["rldst"], "lsb_r")
            wsb_r = wpool.tile([128, rev["NV"]], f32, tag="wgt", name="wsb_r")
            nc.sync.dma_start(out=wsb_r[:], in_=pin["rwgt"][:])
            nc.sync.dma_start(out=xacc_sb[:], in_=xacc[:])
            nc.sync.dma_start(out=ws[:], in_=wsum4[:])
            nc.sync.dma_start(out=wb[:], in_=wblk4.rearrange("r p n -> p r n"))
            nc.sync.dma_start(out=nb[:], in_=bcat[:])
            nc.sync.dma_start(out=ngb[:], in_=nbneg[:])
            nc.sync.dma_start(out=lw[:], in_=linw[:])
            nc.sync.dma_start(out=lb[:], in_=linb[:])
            nc.sync.dma_start(out=rpo_sb[:], in_=rpwo[:])
            nc.sync.dma_start(out=rpi_sb[:], in_=rpwi[:])

            for k in range(NSEG):
                drain_q(cpool, psumt, accs[k], k,
                        to1_sb[:, J0[k] * 128:(J0[k] + JSPLIT[k]) * 128],
                        rpo_sb, sc, t1o_b, identbf)
            nc.gpsimd.collective_compute(
                "AllGather", OP.bypass, replica_groups=RG,
                ins=[t1o_b.opt()], outs=[t1o_tab.opt()],
            )

            # ---------------- pass 2: hop1 rev ----------------
            accs = hop_pass(pool, psum, rev, isb_r, lsb_r, wsb_r, xbf, iota)
            for k in range(NSEG):
                drain_q(cpool, psumt, accs[k], k,
                        ti1_sb[:, J0[k] * 128:(J0[k] + JSPLIT[k]) * 128],
                        rpi_sb, sc, t1i_b, identbf)
            nc.gpsimd.collective_compute(
                "AllGather", OP.bypass, replica_groups=RG,
                ins=[t1i_b.opt()], outs=[t1i_tab.opt()],
            )

            # ---------------- pass 3: hop2 fwd ----------------
            accs = hop_pass(pool, psum, fwd, isb_f, lsb_f, None, t1o_tab, iota)
            for k in range(NSEG):
                drain_q(cpool, psumt, accs[k], k,
                        to2_sb[:, J0[k] * 128:(J0[k] + JSPLIT[k]) * 128],
                        None, None, None, identbf)

            # ---------------- pass 4: hop2 rev + gates -------------------
            # quarter drains land in ti2_k[k]; gate tiles for quarter k start
            # as soon as it drains, hiding most of the gate phase under the
            # hop2r gather stream.
            outv = outT.rearrange("c (j q p) -> q c j p", q=4, p=128)
            per = CALL // 128
            qlast = {}
            for e in rev["chunks"]:
                kq = quarter_of(e[2] // 4)
                qlast[kq] = max(qlast.get(kq, -1), e[0] // per)
            ncalls_r = rev["S"] // CALL

            def emit_quarter(k, accs):
                nc.vector.tensor_copy(out=ti2_k[k][:], in_=accs[k][:])
                j0, jn = J0[k], JSPLIT[k]
                for qi in range(4):
                    sl4 = slice(32 * qi, 32 * qi + 32)
                    ot = pool.tile([OUTC, max(QCOL)], f32, tag="g_ot", bufs=2, name="ot")
                    for c0, TW in tiles_of(jn):
                        gsl = slice(j0 * 128 + c0, j0 * 128 + c0 + TW)
                        G = psg.tile([128, 512], f32, space="PSUM", tag="G",
                                     bufs=2, name="G")
                        terms = [
                            (ws[sl4, :], xacc_sb[sl4, gsl]),
                            (wb[sl4, 0, :], to1_sb[sl4, gsl]),
                            (wb[sl4, 1, :], ti1_sb[sl4, gsl]),
                            (wb[sl4, 2, :], to2_sb[sl4, gsl]),
                            (wb[sl4, 3, :], ti2_k[k][sl4, c0:c0 + TW]),
                        ]
                        for ti, (lhsT, rhs) in enumerate(terms):
                            nc.tensor.matmul(
                                G[:, 0:TW], lhsT=lhsT, rhs=rhs,
                                start=(ti == 0), stop=(ti == len(terms) - 1),
                                tile_position=(32 * qi, 0),
                            )
                        zb = pool.tile([GATE, 512], bf16, tag="g_zb", name="zb")
                        ht = pool.tile([GATE, 512], bf16, tag="g_ht", name="ht")
                        nc.scalar.activation(
                            out=zb[:, 0:TW], in_=G[0:GATE, 0:TW], func=AF.Sigmoid,
                            bias=ngb[0:GATE, :], scale=-1.0,
                        )
                        nc.scalar.activation(
                            out=ht[:, 0:TW], in_=G[GATE:128, 0:TW], func=AF.Tanh,
                            bias=nb[GATE:128, :], scale=1.0,
                        )
                        hs = pool.tile([GATE, 512], bf16, tag="g_hs", name="hs")
                        nc.vector.tensor_tensor(
                            out=hs[:, 0:TW], in0=zb[:, 0:TW], in1=ht[:, 0:TW],
                            op=OP.mult,
                        )
                        hr = pool.tile([GATE, 512], bf16, tag="g_hr", name="hr")
                        nc.vector.tensor_scalar(
                            out=hr[:, 0:TW], in0=hs[:, 0:TW], scalar1=0.0,
                            scalar2=None, op0=OP.max,
                        )
                        po = psg.tile([OUTC, 512], f32, space="PSUM", tag="po",
                                      bufs=1, name="po")
                        nc.tensor.matmul(
                            po[:, 0:TW], lhsT=lw[:], rhs=hr[:, 0:TW],
                            start=True, stop=True,
                        )
                        nc.vector.tensor_scalar(
                            out=ot[:, c0:c0 + TW], in0=po[:, 0:TW], scalar1=lb[:],
                            scalar2=None, op0=OP.add,
                        )
                    nc.sync.dma_start(
                        out=outv[qi][:, j0:j0 + jn, :],
                        in_=ot[:, 0:jn * 128].rearrange("c (j p) -> c j p", p=128),
                    )

            done = set()

            def on_call_end(call, accs):
                for k in range(NSEG):
                    if qlast.get(k) == call and k not in done:
                        done.add(k)
                        emit_quarter(k, accs)

            hop_pass(pool, psum, rev, isb_r, lsb_r, None, t1i_tab, iota,
                     on_call_end=on_call_end)
            for k in range(NSEG):
                if k not in done:
                    raise AssertionError(f"quarter {k} never completed")

    nc.compile()
    return nc


_CACHE = {}
_JIT = {}


def _get_built(edge_index, edge_weight):
    x0 = np.zeros((N, C), np.float32)
    npad, sh, fwd, rev, rpw_o, rpw_i = _host_prep(x0, edge_index, edge_weight)
    nc = _build(npad, sh, fwd, rev)
    return npad, sh, fwd, rev, rpw_o, rpw_i, nc


def _run_spmd_cached(nc, in_maps):
    """run_bass_kernel_spmd's axon path with the jitted executable cached
    across calls (the library re-jits a fresh closure per call, costing ~2s
    of retrace + BIR verify on every invocation)."""
    import jax
    import numpy as _np
    from jax.sharding import Mesh, PartitionSpec
    from jax.experimental.shard_map import shard_map
    import concourse.mybir as mybir
    from concourse import bass2jax

    n_cores = NCORES
    key = id(nc)
    if key not in _JIT:
        bass2jax.install_neuronx_cc_hook()
        pname = nc.partition_id_tensor.name if nc.partition_id_tensor else None
        in_names, out_names, out_avals, zero_shapes = [], [], [], []
        for alloc in nc.m.functions[0].allocations:
            if not isinstance(alloc, mybir.MemoryLocationSet):
                continue
            name = alloc.memorylocations[0].name
            if alloc.kind == "ExternalInput":
                if name != pname:
                    in_names.append(name)
            elif alloc.kind == "ExternalOutput":
                out_names.append(name)
                shape = tuple(alloc.tensor_shape)
                dtype = mybir.dt.np(alloc.dtype)
                out_avals.append(jax.core.ShapedArray(shape, dtype))
                zero_shapes.append((shape, dtype))
        n_params = len(in_names)
        all_names = in_names + out_names
        if pname is not None:
            all_names = all_names + [pname]

        def _body(*args):
            operands = list(args)
            if pname is not None:
                operands.append(bass2jax.partition_id_tensor())
            outs = bass2jax._bass_exec_p.bind(
                *operands,
                out_avals=tuple(out_avals),
                in_names=tuple(all_names),
                out_names=tuple(out_names),
                lowering_input_output_aliases=(),
                sim_require_finite=True,
                sim_require_nnan=True,
                nc=nc,
            )
            return tuple(outs)

        devices = jax.devices()[:n_cores]
        mesh = Mesh(_np.asarray(devices), ("core",))
        n_outs = len(out_names)
        sharded = jax.jit(
            shard_map(
                _body, mesh=mesh,
                in_specs=(PartitionSpec("core"),) * (n_params + n_outs),
                out_specs=(PartitionSpec("core"),) * n_outs,
                check_rep=False,
            ),
            donate_argnums=tuple(range(n_params, n_params + n_outs)),
            keep_unused=True,
        )
        _JIT.clear()
        _JIT[key] = (sharded, in_names, out_names, out_avals, zero_shapes)
    sharded, in_names, out_names, out_avals, zero_shapes = _JIT[key]

    # graph-constant tables live on device across calls; only x/weight-derived
    # inputs are transferred per call
    STATIC = {"fidx", "fldst", "fwgt", "ridx", "rldst", "rwgt",
              "io_bf", "idbf", "rpwo", "rpwi"}
    skey = (key, "static")
    if skey not in _JIT:
        from jax.sharding import NamedSharding
        devices = jax.devices()[:n_cores]
        mesh = Mesh(_np.asarray(devices), ("core",))
        sh_spec = NamedSharding(mesh, PartitionSpec("core"))
        statics = {}
        for name in in_names:
            if name in STATIC:
                arr = np.concatenate([np.asarray(m[name]) for m in in_maps], axis=0)
                statics[name] = jax.device_put(arr, sh_spec)
        _JIT[skey] = statics
    statics = _JIT[skey]

    concat_in = [
        statics[name] if name in statics else
        np.concatenate([np.asarray(m[name]) for m in in_maps], axis=0)
        for name in in_names
    ]
    concat_zeros = [
        np.zeros((n_cores * s[0], *s[1:]), d) for s, d in zero_shapes
    ]
    out_arrs = sharded(*concat_in, *concat_zeros)
    return [
        {
            name: np.asarray(out_arrs[i]).reshape(n_cores, *out_avals[i].shape)[c]
            for i, name in enumerate(out_names)
        }
        for c in range(n_cores)
    ]


def kernel(x, edge_index, edge_weight, w_z, b_z, w_r, b_r, w_h, b_h, lin_w, lin_b):
    import ml_dtypes
    from concourse.bass_utils import run_bass_kernel_spmd

    x = np.asarray(x, np.float32)
    edge_index = np.asarray(edge_index)
    edge_weight = np.asarray(edge_weight, np.float32)
    import hashlib
    key = hashlib.sha1(
        np.ascontiguousarray(edge_index).tobytes()
        + np.ascontiguousarray(edge_weight).tobytes()
    ).hexdigest()
    if key not in _CACHE:
        _CACHE.clear()
        _CACHE[key] = _get_built(edge_index, edge_weight)
    npad, sh, fwd, rev, rpw_o, rpw_i, nc = _CACHE[key]
    JW = sh // 512

    x_pad = np.zeros((npad, C), np.float32)
    x_pad[:N] = x
    xbf = x_pad.astype(ml_dtypes.bfloat16)

    wz = np.asarray(w_z, np.float32)
    wh = np.asarray(w_h, np.float32)

    def blk(d, k):
        b = np.zeros((C, 128), np.float32)
        b[:, 0:GATE] = wz[d, k, :C]
        b[:, GATE:128] = wh[d, k, :C]
        return b

    wsum4 = np.tile(blk(0, 0) + blk(1, 0), (4, 1))
    pairs = [(0, 1), (1, 1), (0, 2), (1, 2)]  # to1, ti1, to2, ti2 terms
    wblk4 = np.stack([np.tile(blk(d, k), (4, 1)) for d, k in pairs])
    bcat = np.concatenate(
        [np.asarray(b_z, np.float32), np.asarray(b_h, np.float32)]
    ).reshape(128, 1)
    nbneg = -bcat

    iota = np.tile(np.arange(128, dtype=np.float32), (128, 1))
    base = {
        "xbf": xbf,
        "io_bf": iota.astype(ml_dtypes.bfloat16),
        "idbf": np.eye(128, dtype=np.float32).astype(ml_dtypes.bfloat16),
        "wsum4": wsum4.astype(ml_dtypes.bfloat16),
        "wblk4": wblk4.astype(ml_dtypes.bfloat16),
        "bcat": bcat,
        "nbneg": nbneg,
        "linw": np.asarray(lin_w, np.float32).astype(ml_dtypes.bfloat16),
        "linb": np.asarray(lin_b, np.float32).reshape(OUTC, 1),
    }
    in_maps = []
    for c in range(NCORES):
        m = dict(base)
        m["xacc"] = np.ascontiguousarray(
            x_pad[c * sh:(c + 1) * sh]
            .reshape(JW, 4, 128, C).transpose(1, 3, 0, 2).reshape(128, JW * 128)
        ).astype(ml_dtypes.bfloat16)
        m["fidx"] = fwd["gidx"][c]
        m["fldst"] = fwd["ldst"][c]
        m["fwgt"] = fwd["wgt"][c]
        m["ridx"] = rev["gidx"][c]
        m["rldst"] = rev["ldst"][c]
        m["rwgt"] = rev["wgt"][c]
        m["rpwo"] = rpw_o[c]
        m["rpwi"] = rpw_i[c]
        in_maps.append(m)

    import os
    trace = bool(int(os.environ.get("DCRNN_TRACE", "0")))
    global LAST_EXEC_NS
    if trace:
        res = run_bass_kernel_spmd(
            nc, in_maps, core_ids=list(range(NCORES)), trace=True
        )
        LAST_EXEC_NS = res.exec_time_ns
        results = res.results
    else:
        LAST_EXEC_NS = None
        results = _run_spmd_cached(nc, in_maps)
    out = np.concatenate([results[c]["outT"] for c in range(NCORES)], axis=1)
    return np.ascontiguousarray(out.T[:N]).astype(np.float32)
